# revision 13
# baseline (speedup 1.0000x reference)
"""CLIP vision transformer (prompt tuning variant) on 8 TRN2 NeuronCores.

Data-parallel over batch (64 images -> 8 per core), weights replicated.
Per core the full 12-layer ViT runs on-device:
  - residual stream x: token-major fp32 in SBUF, per-image tiling
    (each image: one 128-row tile + one 77-row tile; token order is
    196 patch tokens, then CLS, then 8 prompt tokens)
  - projections (QKV/O) in float32r (FP22, full PE rate at N>=256)
  - attention QK^T / AV and the whole MLP in bf16 (fp32 psum accum)
  - softmax without max-subtraction (scores are O(1) here); the
    normalizer z comes from an extra ones-column appended per head in V,
    so the AV matmul emits z as psum column 64 -> per-partition
    reciprocal, no cross-partition reduction
  - ln1/ln2 gamma/beta folded into the following projection weights;
    pre/post LN applied via host-prepared broadcast tiles
"""

import numpy as np
import ml_dtypes

import concourse.bass as bass
import concourse.mybir as mybir
import concourse.tile as tile
from concourse import bass_utils
from concourse.vector_clock import ScopedClock

F32 = mybir.dt.float32
F32R = mybir.dt.float32r
BF16 = mybir.dt.bfloat16
AF = mybir.ActivationFunctionType
ALU = mybir.AluOpType
AX = mybir.AxisListType

D, H, DH, L, FF, P, IMG, NPR = 768, 12, 64, 12, 3072, 16, 224, 8
SCALE = 0.125
B_TOT, N_CORES = 64, 8
B = B_TOT // N_CORES          # images per core (8)
S = 205                       # tokens per image
SP = 208                      # padded free-dim stride per image
TF = B * SP                   # feature-major free width (1664)
NPATCH = 196
CLS_ROW = 68                  # cls row within tile 1 (= 196-128)
IMG_TILES = [(0, 128), (1, 77)]
DT = D // 128                 # 6
FT = FF // 128                # 24
VHW = DH + 1                  # v head stride incl. ones column (65)
EPS = 1e-5
TS = bass.ts


# ---------------------------------------------------------------------------
# walrus here accepts at most one semaphore wait per instruction; split
# extras onto same-engine NOPs (and same for the TileContext teardown drain).
# ---------------------------------------------------------------------------
def _split_sync_waits(nc, max_waits=1):
    n = 0
    for f in nc.m.functions:
        for bb in f.blocks:
            out = []
            for inst in bb.instructions:
                si = inst.sync_info
                waits = list(si.on_wait) if si and si.on_wait else []
                if len(waits) > max_waits:
                    for i in range(0, len(waits) - max_waits, max_waits):
                        nop = mybir.InstNoOp(name=f"I-wsplit{n}", ins=[], outs=[])
                        n += 1
                        nop.engine = inst.engine
                        nop.sync_info = mybir.SyncInfo(
                            on_wait=waits[i:i + max_waits], on_update=[])
                        out.append(nop)
                    si.on_wait = waits[len(waits) - max_waits:]
                out.append(inst)
            bb.instructions = out
    return n


def _patched_drain_and_barrier(self, tick_clock, wait_clock):
    nc = self.nc
    drain_inst = nc.sync.drain()
    wait_clock.add_sem_waits(
        drain_inst.ins, ScopedClock({None: tick_clock.global_clock}))
    si = drain_inst.ins.sync_info
    waits = list(si.on_wait or [])
    if len(waits) > 1:
        si.on_wait = waits[:1]
        for i in range(1, len(waits)):
            nop = nc.sync.nop(nofuse=True, hint=f"drain_w{i}")
            nop.ins.sync_info = mybir.SyncInfo(on_wait=[waits[i]], on_update=[])
    nc.all_engine_barrier()
    assert self.sems is not None
    popped = nc._tile_sem_poison_stack.pop()
    assert popped is self._sem_poison
    nc.clear_and_free_semaphores(list(self.sems.allocated().values()))
    nc.all_engine_barrier()


tile.TileContext._drain_and_barrier = _patched_drain_and_barrier


# ---------------------------------------------------------------------------
# host-side input preparation
# ---------------------------------------------------------------------------
def _prepare(inputs):
    f32 = np.float32
    bf = ml_dtypes.bfloat16
    im = np.asarray(inputs["image"], f32)
    gp = np.asarray(inputs["g_prompt"], f32)
    patch_w = np.asarray(inputs["patch_w"], f32)
    cls_emb = np.asarray(inputs["cls_emb"], f32)
    pos_emb = np.asarray(inputs["pos_emb"], f32)

    # im2col -> feature-major patches, padded to SP columns per image
    p = im.reshape(B_TOT, 3, 14, P, 14, P).transpose(0, 2, 4, 1, 3, 5)
    p = p.reshape(B_TOT, NPATCH, 3 * P * P)
    patches_fm = np.zeros((B_TOT, D, SP), bf)
    patches_fm[:, :, :NPATCH] = p.transpose(0, 2, 1).astype(bf)

    patch_wT = patch_w.reshape(D, 3 * P * P).T.astype(bf)      # [768in, 768out]

    extra0 = np.zeros((B_TOT, 1 + NPR, D), f32)
    extra0[:, 0] = cls_emb + pos_emb[0]
    extra0[:, 1:] = gp[:, 0]

    pos_add = np.zeros((2, 128, D), f32)
    pos_add[0] = pos_emb[1:129]
    pos_add[1, :NPATCH - 128] = pos_emb[129:1 + NPATCH]

    pre_gb = np.stack([
        np.broadcast_to(np.asarray(inputs["pre_g"], f32), (128, D)),
        np.broadcast_to(np.asarray(inputs["pre_b"], f32), (128, D)),
    ]).copy()
    post_gb = np.stack([
        np.broadcast_to(np.asarray(inputs["post_g"], f32), (B, D)),
        np.broadcast_to(np.asarray(inputs["post_b"], f32), (B, D)),
    ]).copy()

    qw = np.asarray(inputs["qw"], f32); qb = np.asarray(inputs["qb"], f32)
    kw = np.asarray(inputs["kw"], f32); kb = np.asarray(inputs["kb"], f32)
    vw = np.asarray(inputs["vw"], f32); vb = np.asarray(inputs["vb"], f32)
    ow = np.asarray(inputs["ow"], f32); ob = np.asarray(inputs["ob"], f32)
    g1 = np.asarray(inputs["ln1_g"], f32); b1 = np.asarray(inputs["ln1_b"], f32)
    g2 = np.asarray(inputs["ln2_g"], f32); b2 = np.asarray(inputs["ln2_b"], f32)
    f1w = np.asarray(inputs["fc1_w"], f32); f1b = np.asarray(inputs["fc1_b"], f32)
    f2w = np.asarray(inputs["fc2_w"], f32); f2b = np.asarray(inputs["fc2_b"], f32)

    # fold ln1 into qkv, ln2 into fc1   (w[l,o,d] -> wT[l,d,o])
    qwT = ((qw * g1[:, None, :]).transpose(0, 2, 1) * SCALE).astype(bf)
    kwT = (kw * g1[:, None, :]).transpose(0, 2, 1).astype(bf)
    vwT = (vw * g1[:, None, :]).transpose(0, 2, 1).astype(bf)
    owT = ow.transpose(0, 2, 1).astype(bf)
    f1wT = (f1w * g2[:, None, :]).transpose(0, 2, 1).astype(bf)  # [12,768,3072]
    f2wT = f2w.transpose(0, 2, 1).astype(bf)                     # [12,3072,768]

    qb_eff = (qb + np.einsum("lod,ld->lo", qw, b1)) * SCALE
    kb_eff = kb + np.einsum("lod,ld->lo", kw, b1)
    vb_eff = vb + np.einsum("lod,ld->lo", vw, b1)
    f1b_eff = f1b + np.einsum("lfd,ld->lf", f1w, b2)

    qk_cols = np.concatenate([
        qb_eff.reshape(L, DT, 128).transpose(0, 2, 1),
        kb_eff.reshape(L, DT, 128).transpose(0, 2, 1)], axis=2).copy()
    f1b_cols = f1b_eff.reshape(L, FT, 128).transpose(0, 2, 1).copy()  # [L,128,24]
    bias_rows = np.stack([vb_eff, ob, f2b], axis=1).astype(bf)        # [L, 3, 768]

    per_core = {"patches_fm": patches_fm, "extra0": extra0, "gp": gp}
    shared = {
        "patch_wT": patch_wT, "pos_add": pos_add, "pre_gb": pre_gb,
        "post_gb": post_gb, "qwT": qwT, "kwT": kwT, "vwT": vwT, "owT": owT,
        "f1wT": f1wT, "f2wT": f2wT, "qk_cols": qk_cols,
        "f1b_cols": f1b_cols, "bias_rows": bias_rows,
        "ones_row": np.ones((1, 128), bf), "ident": np.eye(128, dtype=bf),
    }
    return per_core, shared


_DRAM_SPECS = {
    "patches_fm": ((B, D, SP), BF16),
    "extra0": ((B, 1 + NPR, D), F32),
    "gp": ((B, L, NPR, D), F32),
    "patch_wT": ((D, D), BF16),
    "pos_add": ((2, 128, D), F32),
    "pre_gb": ((2, 128, D), F32),
    "post_gb": ((2, B, D), F32),
    "qwT": ((L, D, D), BF16),
    "kwT": ((L, D, D), BF16),
    "vwT": ((L, D, D), BF16),
    "owT": ((L, D, D), BF16),
    "f1wT": ((L, D, FF), BF16),
    "f2wT": ((L, FF, D), BF16),
    "qk_cols": ((L, 128, 2 * DT), F32),
    "f1b_cols": ((L, 128, FT), F32),
    "bias_rows": ((L, 3, D), BF16),
    "ones_row": ((1, 128), BF16),
    "ident": ((128, 128), BF16),
}

N2 = [(0, 512), (512, 256)]    # 768-wide N split


# ---------------------------------------------------------------------------
# device program
# ---------------------------------------------------------------------------
def _ln_stats(nc, sp, x_ap, scratch_ap, nrows, tag):
    """mean/rstd columns ([128,1] tiles, first nrows valid) for rowwise LN."""
    s1 = sp.tile([128, 1], F32, tag=f"{tag}s1", name=f"{tag}s1")
    s2 = sp.tile([128, 1], F32, tag=f"{tag}s2", name=f"{tag}s2")
    mu = sp.tile([128, 1], F32, tag=f"{tag}mu", name=f"{tag}mu")
    var = sp.tile([128, 1], F32, tag=f"{tag}var", name=f"{tag}var")
    rstd = sp.tile([128, 1], F32, tag=f"{tag}rstd", name=f"{tag}rstd")
    r = slice(0, nrows)
    nc.vector.reduce_sum(s1[r, :], x_ap, axis=AX.X)
    nc.scalar.activation(scratch_ap, x_ap, AF.Square, accum_out=s2[r, :])
    nc.vector.tensor_scalar_mul(mu[r, :], s1[r, :], 1.0 / D)
    nc.vector.tensor_mul(var[r, :], mu[r, :], mu[r, :])          # mu^2
    nc.vector.tensor_scalar(s2[r, :], s2[r, :], 1.0 / D, None, op0=ALU.mult)
    nc.vector.tensor_sub(var[r, :], s2[r, :], var[r, :])         # E[x^2]-mu^2
    nc.vector.tensor_scalar(var[r, :], var[r, :], EPS, None, op0=ALU.add)
    nc.scalar.activation(var[r, :], var[r, :], AF.Sqrt)
    nc.vector.reciprocal(rstd[r, :], var[r, :])
    return mu, rstd


def _ln_chunk(nc, lp, sp, psp, x, h_fm, ident, ch, tag, out_dt):
    """LN (stats only) + transpose for one chunk (2 images / 4 tiles).

    Stats for the 4 tiles are batched into [128, 4] column tiles so the
    whole chunk needs one Sqrt + one reciprocal (tames ACT-table swaps
    and tiny-op overhead)."""
    s1 = sp.tile([128, 4], F32, tag=f"{tag}s1", name=f"{tag}s1")
    s2 = sp.tile([128, 4], F32, tag=f"{tag}s2", name=f"{tag}s2")
    mu = sp.tile([128, 4], F32, tag=f"{tag}mu", name=f"{tag}mu")
    var = sp.tile([128, 4], F32, tag=f"{tag}var", name=f"{tag}var")
    rstd = sp.tile([128, 4], F32, tag=f"{tag}rs", name=f"{tag}rs")
    htoks = []
    for t in range(4):
        img, it = ch * 2 + t // 2, t % 2
        i = img * 2 + it
        nrows = IMG_TILES[it][1]
        r = slice(0, nrows)
        htok = lp.tile([128, D], out_dt, tag=f"htok{t}", name=f"htok{t}")
        scr = lp.tile([128, D], F32, tag="hscr", name="hscr")
        nc.vector.reduce_sum(s1[r, t:t + 1], x[i][r, :], axis=AX.X)
        nc.scalar.activation(scr[r, :], x[i][r, :], AF.Square,
                             accum_out=s2[r, t:t + 1])
        htoks.append((htok, i, nrows))
    nc.vector.tensor_scalar_mul(mu[:, :], s1[:, :], 1.0 / D)
    nc.vector.tensor_mul(var[:, :], mu[:, :], mu[:, :])
    nc.vector.tensor_scalar(s2[:, :], s2[:, :], 1.0 / D, None, op0=ALU.mult)
    nc.vector.tensor_sub(var[:, :], s2[:, :], var[:, :])
    nc.vector.tensor_scalar(var[:, :], var[:, :], EPS, None, op0=ALU.add)
    nc.scalar.activation(var[:, :], var[:, :], AF.Sqrt)
    nc.vector.reciprocal(rstd[:, :], var[:, :])
    for t, (htok, i, nrows) in enumerate(htoks):
        r = slice(0, nrows)
        nc.vector.tensor_scalar(htok[r, :], x[i][r, :], mu[r, t:t + 1],
                                rstd[r, t:t + 1], op0=ALU.subtract,
                                op1=ALU.mult)
    for t, (htok, i, nrows) in enumerate(htoks):
        img, it = i // 2, i % 2
        c0 = (img % 2) * SP + it * 128
        for k in range(DT):
            ps = psp.tile([128, 128], out_dt, tag="tp", name="tp")
            nc.tensor.transpose(ps[:, :], htok[:, TS(k, 128)], ident[:, :])
            nc.vector.tensor_copy(h_fm[k][ch][:, c0:c0 + nrows],
                                  ps[:, :nrows])


def _build(nc, tc, dr, out):
    # persistent residual stream: 16 tiles [128, 768] fp32
    xp = tc.alloc_tile_pool(name="x", bufs=1)
    x = [xp.tile([128, D], F32, tag=f"x{i}", name=f"x{i}") for i in range(2 * B)]

    cp = tc.alloc_tile_pool(name="const", bufs=1)
    ident = cp.tile([128, 128], BF16, tag="ident", name="ident")
    ones_r = cp.tile([1, 128], BF16, tag="ones", name="ones")
    nc.sync.dma_start(ident[:, :], dr["ident"][:, :])
    nc.sync.dma_start(ones_r[:, :], dr["ones_row"][:, :])

    # ---------------- embedding ----------------
    with tc.tile_pool(name="emb", bufs=2) as ep, \
         tc.tile_pool(name="embw", bufs=1) as ewp, \
         tc.tile_pool(name="embps", bufs=3, space="PSUM") as psp:
        pw = [ewp.tile([128, D], BF16, tag=f"pw{k}", name=f"pw{k}") for k in range(DT)]
        for k in range(DT):
            nc.sync.dma_start(pw[k][:, :], dr["patch_wT"][TS(k, 128), :])
        pos = [ewp.tile([128, D], F32, tag=f"pos{t}", name=f"pos{t}") for t in range(2)]
        for t in range(2):
            nc.sync.dma_start(pos[t][:, :], dr["pos_add"][t, :, :])
        for img in range(B):
            pat = [ep.tile([128, SP], BF16, tag=f"pat{k}", name=f"pat{k}") for k in range(DT)]
            for k in range(DT):
                nc.sync.dma_start(pat[k][:, :], dr["patches_fm"][img, TS(k, 128), :])
            for it, nrows in IMG_TILES:
                xt = x[img * 2 + it]
                nrp = nrows if it == 0 else NPATCH - 128   # patch rows here
                ps = psp.tile([128, D], F32, tag="embps", name="embps")
                for n0, nw in N2:
                    for k in range(DT):
                        nc.tensor.matmul(
                            ps[:nrp, n0:n0 + nw],
                            pat[k][:, it * 128: it * 128 + nrp],
                            pw[k][:, n0:n0 + nw],
                            start=(k == 0), stop=(k == DT - 1))
                nc.vector.tensor_add(xt[:nrp, :], ps[:nrp, :], pos[it][:nrp, :])
            nc.sync.dma_start(x[img * 2 + 1][CLS_ROW:CLS_ROW + 1 + NPR, :],
                              dr["extra0"][img, :, :])

    # ---------------- pre-layernorm (with real gamma/beta) ----------------
    with tc.tile_pool(name="preln", bufs=3) as pp, \
         tc.tile_pool(name="prest", bufs=1) as sp, \
         tc.tile_pool(name="pregb", bufs=1) as gbp:
        g_bc = gbp.tile([128, D], F32, tag="g_bc", name="g_bc")
        b_bc = gbp.tile([128, D], F32, tag="b_bc", name="b_bc")
        nc.sync.dma_start(g_bc[:, :], dr["pre_gb"][0, :, :])
        nc.sync.dma_start(b_bc[:, :], dr["pre_gb"][1, :, :])
        for i in range(2 * B):
            nrows = IMG_TILES[i % 2][1]
            r = slice(0, nrows)
            scr = pp.tile([128, D], F32, tag="pscr", name="pscr")
            mu, rstd = _ln_stats(nc, sp, x[i][r, :], scr[r, :], nrows, f"p{i}")
            nc.vector.tensor_scalar(x[i][r, :], x[i][r, :], mu[r, :], rstd[r, :],
                                    op0=ALU.subtract, op1=ALU.mult)
            nc.vector.tensor_mul(x[i][r, :], x[i][r, :], g_bc[r, :])
            nc.vector.tensor_add(x[i][r, :], x[i][r, :], b_bc[r, :])

    import os
    nlayers = int(os.environ.get("KBENCH_LAYERS", L))
    for l in range(nlayers):
        _layer(nc, tc, dr, x, ident, ones_r, l)

    # ---------------- final LN on cls rows ----------------
    with tc.tile_pool(name="fin", bufs=1) as fp, \
         tc.tile_pool(name="finst", bufs=1) as fsp:
        cls = fp.tile([B, D], F32, tag="cls", name="cls")
        for img in range(B):
            nc.sync.dma_start(cls[img:img + 1, :],
                              x[img * 2 + 1][CLS_ROW:CLS_ROW + 1, :])
        scr = fp.tile([B, D], F32, tag="fscr", name="fscr")
        mu, rstd = _ln_stats(nc, fsp, cls[:, :], scr[:, :], B, "f")
        nc.vector.tensor_scalar(cls[:, :], cls[:, :], mu[:B, :], rstd[:B, :],
                                op0=ALU.subtract, op1=ALU.mult)
        g_bc = fp.tile([B, D], F32, tag="fg", name="fg")
        b_bc = fp.tile([B, D], F32, tag="fb", name="fb")
        nc.sync.dma_start(g_bc[:, :], dr["post_gb"][0, :, :])
        nc.sync.dma_start(b_bc[:, :], dr["post_gb"][1, :, :])
        nc.vector.tensor_mul(cls[:, :], cls[:, :], g_bc[:, :])
        nc.vector.tensor_add(cls[:, :], cls[:, :], b_bc[:, :])
        nc.sync.dma_start(out[:, :], cls[:, :])
    cp.release()
    xp.release()


def _layer(nc, tc, dr, x, ident, ones_r, l):
    NCH = 4
    CW = TF // NCH             # 416

    if l > 0:
        for img in range(B):
            nc.sync.dma_start(
                x[img * 2 + 1][CLS_ROW + 1:CLS_ROW + 1 + NPR, :],
                dr["gp"][img, l, :, :])

    # layer-long feature-major pool; slots are reused (same tags) for
    # h1_fm -> a_fm -> h2_fm
    fmp = tc.alloc_tile_pool(name=f"fm{l}", bufs=1)
    h_fm = [[fmp.tile([128, CW], BF16, tag=f"hfm{k}_{c}", name=f"hfm{k}_{c}")
             for c in range(NCH)] for k in range(DT)]

    qkvp = tc.alloc_tile_pool(name=f"qkv{l}", bufs=1)
    q_sb = [qkvp.tile([128, TF], BF16, tag=f"q{k}", name=f"q{k}")
            for k in range(DT)]
    k_sb = [qkvp.tile([128, TF], BF16, tag=f"k{k}", name=f"k{k}")
            for k in range(DT)]
    v_sb = [qkvp.tile([128, H * VHW], BF16, tag=f"v{i}", name=f"v{i}")
            for i in range(2 * B)]

    # ---- attention weights: whole layer loaded once ----
    awp = tc.alloc_tile_pool(name=f"aw{l}", bufs=1)
    wq = [awp.tile([128, D], BF16, tag=f"wq{dk}", name=f"wq{dk}")
          for dk in range(DT)]
    wk = [awp.tile([128, D], BF16, tag=f"wk{dk}", name=f"wk{dk}")
          for dk in range(DT)]
    vw = [awp.tile([128, D], BF16, tag=f"vw{dk}", name=f"vw{dk}")
          for dk in range(DT)]
    for dk in range(DT):
        nc.sync.dma_start(wq[dk][:, :], dr["qwT"][l, TS(dk, 128), :])
        nc.sync.dma_start(wk[dk][:, :], dr["kwT"][l, TS(dk, 128), :])
        nc.sync.dma_start(vw[dk][:, :], dr["vwT"][l, TS(dk, 128), :])
    qkb = awp.tile([128, 2 * DT], F32, tag="qkb", name="qkb")
    nc.sync.dma_start(qkb[:, :], dr["qk_cols"][l, :, :])
    vbr = awp.tile([1, D], BF16, tag="vbr", name="vbr")
    nc.sync.dma_start(vbr[:, :], dr["bias_rows"][l, 0:1, :])

    # ======== phase A: LN1 + QKV, chunk-major (PE stays dense) ========
    with tc.tile_pool(name=f"lna{l}", bufs=2) as lp, \
         tc.tile_pool(name=f"lnast{l}", bufs=2) as sp, \
         tc.tile_pool(name=f"tpa{l}", bufs=2, space="PSUM") as tpp, \
         tc.tile_pool(name=f"qkps{l}", bufs=4, space="PSUM") as qkpsp, \
         tc.tile_pool(name=f"vps{l}", bufs=1, space="PSUM") as vpsp:
        for ch in range(NCH):
            _ln_chunk(nc, lp, sp, tpp, x, h_fm, ident, ch, "a", BF16)
            # qk projections for this chunk
            for wsel, wt, bc0 in ((0, wq, 0), (1, wk, DT)):
                sb = q_sb if wsel == 0 else k_sb
                for o in range(DT):
                    ps = qkpsp.tile([128, CW], F32, tag="qkps", name="qkps")
                    for dk in range(DT):
                        nc.tensor.matmul(ps[:, :], wt[dk][:, TS(o, 128)],
                                         h_fm[dk][ch][:, :],
                                         start=(dk == 0), stop=(dk == DT - 1))
                    nc.vector.tensor_scalar(
                        sb[o][:, TS(ch, CW)], ps[:, :],
                        qkb[:, bc0 + o: bc0 + o + 1], None, op0=ALU.add)
            # v projection for the chunk's 4 tiles (token-major out)
            for t in range(4):
                img, it = ch * 2 + t // 2, t % 2
                i = img * 2 + it
                nrows = IMG_TILES[it][1]
                lc = (img % 2) * SP + it * 128
                ps = vpsp.tile([128, D], F32, tag="vps", name="vps")
                for n0, nw in N2:
                    for dk in range(DT):
                        nc.tensor.matmul(ps[:nrows, n0:n0 + nw],
                                         h_fm[dk][ch][:, lc:lc + nrows],
                                         vw[dk][:, n0:n0 + nw],
                                         start=(dk == 0), stop=False)
                    nc.tensor.matmul(ps[:nrows, n0:n0 + nw], ones_r[:, :nrows],
                                     vbr[:, n0:n0 + nw], start=False, stop=True)
                vt = v_sb[i]
                v3 = vt[:nrows, :].rearrange("p (h w) -> p h w", h=H)
                nc.vector.tensor_copy(
                    v3[:, :, :DH],
                    ps[:nrows, :].rearrange("p (h w) -> p h w", h=H))
                nc.vector.memset(v3[:, :, DH:], 1.0)

    # ======== phase B: attention + O + residual, fused per image ========
    # All 24 QK matmuls of an image are emitted back-to-back; head pairs
    # (2hp, 2hp+1) sit in PE row-halves 0:64 / 64:128 so their matmuls run
    # concurrently in the array.  exp/AV/rz trail behind on other engines.
    with tc.tile_pool(name=f"ow{l}", bufs=1) as owp, \
         tc.tile_pool(name=f"att{l}", bufs=2) as apool, \
         tc.tile_pool(name=f"az{l}", bufs=4) as zpool, \
         tc.tile_pool(name=f"asps{l}", bufs=3, space="PSUM") as spsp, \
         tc.tile_pool(name=f"avps{l}", bufs=2, space="PSUM") as apsp, \
         tc.tile_pool(name=f"atps{l}", bufs=1, space="PSUM") as tpsp, \
         tc.tile_pool(name=f"ops{l}", bufs=1, space="PSUM") as opsp:
        a_fm = [[fmp.tile([128, CW], BF16, tag=f"hfm{k}_{c}", name=f"afm{k}_{c}")
                 for c in range(NCH)] for k in range(DT)]
        ow = [owp.tile([128, D], BF16, tag=f"ow{dk}", name=f"ow{dk}")
              for dk in range(DT)]
        for dk in range(DT):
            nc.sync.dma_start(ow[dk][:, :], dr["owT"][l, TS(dk, 128), :])
        obr = owp.tile([1, D], BF16, tag="obr", name="obr")
        nc.sync.dma_start(obr[:, :], dr["bias_rows"][l, 1:2, :])
        ep = tc.alloc_tile_pool(name=f"ae{l}", bufs=1)
        for img in range(B):
            i0 = img * SP
            a_t = [apool.tile([128, D], BF16, tag="a", name="a")
                   for _ in range(2)]
            e_cur = [[None] * 2 for _ in range(H)]
            # -- all QK score matmuls, head pairs adjacent --
            for hp in range(DT):
                for jt, (jti, jw) in enumerate(IMG_TILES):
                    for sub in range(2):
                        h = hp * 2 + sub
                        ko = sub * 64
                        sps = spsp.tile([128, S], F32, tag="sps", name="sps")
                        nc.tensor.matmul(
                            sps[:jw, :],
                            k_sb[hp][ko:ko + DH,
                                     i0 + jti * 128: i0 + jti * 128 + jw],
                            q_sb[hp][ko:ko + DH, i0:i0 + S],
                            start=True, stop=True)
                        e = ep.tile([128, S], BF16, tag=f"e{h}_{jt}",
                                    name=f"e{h}_{jt}")
                        nc.scalar.activation(e[:jw, :], sps[:jw, :], AF.Exp)
                        e_cur[h][jt] = (e, jw)
            # -- all AV matmuls --
            for h in range(H):
                for it, (iti, iw) in enumerate(IMG_TILES):
                    aps = apsp.tile([128, VHW], F32, tag="aps", name="aps")
                    for jt, (e, jw) in enumerate(e_cur[h]):
                        nc.tensor.matmul(
                            aps[:iw, :],
                            e[:jw, iti * 128: iti * 128 + iw],
                            v_sb[img * 2 + jt][:jw, h * VHW: h * VHW + VHW],
                            start=(jt == 0), stop=(jt == 1))
                    rz = zpool.tile([128, 1], F32, tag="rz", name="rz")
                    nc.vector.reciprocal(rz[:iw, :], aps[:iw, DH:DH + 1])
                    nc.vector.tensor_scalar_mul(
                        a_t[it][:iw, TS(h, DH)], aps[:iw, :DH], rz[:iw, :])
            # -- transpose into a_fm, then O projection + residual --
            ch = img // 2
            for it, nrows in IMG_TILES:
                c0 = (img % 2) * SP + it * 128
                for k in range(DT):
                    ps = tpsp.tile([128, 128], BF16, tag="atp", name="atp")
                    nc.tensor.transpose(ps[:, :], a_t[it][:, TS(k, 128)],
                                        ident[:, :])
                    nc.vector.tensor_copy(a_fm[k][ch][:, c0:c0 + nrows],
                                          ps[:, :nrows])
            for it, nrows in IMG_TILES:
                i = img * 2 + it
                lc = (img % 2) * SP + it * 128
                ps = opsp.tile([128, D], F32, tag="ops", name="ops")
                for n0, nw in N2:
                    for dk in range(DT):
                        nc.tensor.matmul(ps[:nrows, n0:n0 + nw],
                                         a_fm[dk][ch][:, lc:lc + nrows],
                                         ow[dk][:, n0:n0 + nw],
                                         start=(dk == 0), stop=False)
                    nc.tensor.matmul(ps[:nrows, n0:n0 + nw], ones_r[:, :nrows],
                                     obr[:, n0:n0 + nw], start=False, stop=True)
                nc.vector.tensor_add(x[i][:nrows, :], x[i][:nrows, :],
                                     ps[:nrows, :])
        ep.release()
    awp.release()
    qkvp.release()

    # ======== phase C: LN2 + MLP, chunk-major ========
    h2_fm = [[fmp.tile([128, CW], BF16, tag=f"hfm{k}_{c}", name=f"h2fm{k}_{c}")
              for c in range(NCH)] for k in range(DT)]
    PCW = SP                   # 208 (one image per hm round)
    with tc.tile_pool(name=f"m1w{l}", bufs=1) as w1p, \
         tc.tile_pool(name=f"m2w{l}", bufs=1) as w2p, \
         tc.tile_pool(name=f"hm{l}", bufs=1) as hmp, \
         tc.tile_pool(name=f"lnc{l}", bufs=2) as lp, \
         tc.tile_pool(name=f"lncst{l}", bufs=2) as sp, \
         tc.tile_pool(name=f"tpc{l}", bufs=1, space="PSUM") as tpp, \
         tc.tile_pool(name=f"m1ps{l}", bufs=3, space="PSUM") as ps1, \
         tc.tile_pool(name=f"m2ps{l}", bufs=2, space="PSUM") as ps2:
        f1b = w1p.tile([128, FT], F32, tag="f1b", name="f1b")
        nc.sync.dma_start(f1b[:, :], dr["f1b_cols"][l, :, :])
        f2br = w2p.tile([1, D], BF16, tag="f2br", name="f2br")
        nc.sync.dma_start(f2br[:, :], dr["bias_rows"][l, 2:3, :])
        f1w = [w1p.tile([128, FF], BF16, tag=f"f1w{dk}", name=f"f1w{dk}")
               for dk in range(DT)]
        for dk in range(DT):
            nc.sync.dma_start(f1w[dk][:, :], dr["f1wT"][l, TS(dk, 128), :])
        f2w = [w2p.tile([128, D], BF16, tag=f"f2w{fk}", name=f"f2w{fk}")
               for fk in range(FT)]
        for fk in range(FT):
            nc.sync.dma_start(f2w[fk][:, :], dr["f2wT"][l, TS(fk, 128), :])
        for ch in range(NCH):
            _ln_chunk(nc, lp, sp, tpp, x, h2_fm, ident, ch, "c", BF16)
            for sub in range(2):
                img = ch * 2 + sub
                lc0 = (img % 2) * SP
                hm = [hmp.tile([128, PCW], BF16, tag=f"hm{fk}", name=f"hm{fk}",
                               bufs=2)
                      for fk in range(FT)]
                for fk in range(FT):
                    ps = ps1.tile([128, PCW], F32, tag="m1ps", name="m1ps")
                    for dk in range(DT):
                        nc.tensor.matmul(ps[:, :], f1w[dk][:, TS(fk, 128)],
                                         h2_fm[dk][ch][:, lc0:lc0 + PCW],
                                         start=(dk == 0), stop=(dk == DT - 1))
                    nc.scalar.activation(hm[fk][:, :], ps[:, :],
                                         AF.Gelu_apprx_sigmoid,
                                         bias=f1b[:, fk:fk + 1])
                for it, nrows in IMG_TILES:
                    loc = it * 128
                    i = img * 2 + it
                    ps = ps2.tile([128, D], F32, tag="m2ps", name="m2ps")
                    for n0, nw in N2:
                        for fk in range(FT):
                            nc.tensor.matmul(
                                ps[:nrows, n0:n0 + nw],
                                hm[fk][:, loc:loc + nrows],
                                f2w[fk][:, n0:n0 + nw],
                                start=(fk == 0), stop=False)
                        nc.tensor.matmul(ps[:nrows, n0:n0 + nw],
                                         ones_r[:, :nrows],
                                         f2br[:, n0:n0 + nw],
                                         start=False, stop=True)
                    nc.vector.tensor_add(x[i][:nrows, :], x[i][:nrows, :],
                                         ps[:nrows, :])
    fmp.release()


def build_nc():
    nc = bass.Bass(trn_type="TRN2", debug=False)
    dr = {name: nc.dram_tensor(name, list(shape), dt, kind="ExternalInput").ap()
          for name, (shape, dt) in _DRAM_SPECS.items()}
    out = nc.dram_tensor("out", [B, D], F32, kind="ExternalOutput").ap()
    with tile.TileContext(nc, pool_alloc_mode="queue") as tc:
        _build(nc, tc, dr, out)
    _split_sync_waits(nc)
    return nc


# ---------------------------------------------------------------------------
# pjrt runner: shared inputs replicated (sent once), per-core inputs sharded
# ---------------------------------------------------------------------------
_PER_CORE = {"patches_fm", "extra0", "gp"}


def _make_runner(nc):
    import jax
    from jax.sharding import Mesh, PartitionSpec
    from jax.experimental.shard_map import shard_map
    from concourse import bass2jax

    bass2jax.install_neuronx_cc_hook()
    partition_name = (nc.partition_id_tensor.name
                      if nc.partition_id_tensor else None)
    in_names, out_names, out_avals = [], [], []
    for alloc in nc.m.functions[0].allocations:
        if not isinstance(alloc, mybir.MemoryLocationSet):
            continue
        name = alloc.memorylocations[0].name
        if alloc.kind == "ExternalInput":
            if name != partition_name:
                in_names.append(name)
        elif alloc.kind == "ExternalOutput":
            out_names.append(name)
            out_avals.append(jax.core.ShapedArray(
                tuple(alloc.tensor_shape), mybir.dt.np(alloc.dtype)))
    n_params = len(in_names)
    n_outs = len(out_names)
    all_names = in_names + out_names
    if partition_name is not None:
        all_names = all_names + [partition_name]

    def _body(*args):
        operands = list(args)
        if partition_name is not None:
            operands.append(bass2jax.partition_id_tensor())
        outs = bass2jax._bass_exec_p.bind(
            *operands, out_avals=tuple(out_avals), in_names=tuple(all_names),
            out_names=tuple(out_names), lowering_input_output_aliases=(),
            sim_require_finite=True, sim_require_nnan=True, nc=nc)
        return tuple(outs)

    devices = jax.devices()[:N_CORES]
    mesh = Mesh(np.asarray(devices), ("core",))
    Pc, Pr = PartitionSpec("core"), PartitionSpec()
    in_specs = tuple(Pc if n in _PER_CORE else Pr for n in in_names) \
        + (Pc,) * n_outs
    out_specs = (Pc,) * n_outs
    fn = jax.jit(
        shard_map(_body, mesh=mesh, in_specs=in_specs, out_specs=out_specs,
                  check_rep=False),
        donate_argnums=tuple(range(n_params, n_params + n_outs)),
        keep_unused=True)
    return fn, in_names, out_names, out_avals, mesh


# ---------------------------------------------------------------------------
# public entry point
# ---------------------------------------------------------------------------
_CACHED = {}


def _runner():
    if "fn" not in _CACHED:
        nc = build_nc()
        _CACHED["fn"] = _make_runner(nc)
    return _CACHED["fn"]


def _global_args(inputs):
    """Argument list for the jitted fn (per-core tensors carry the full
    batch on axis 0; shared tensors passed once)."""
    per_core, shared = _prepare(inputs)
    fn, in_names, out_names, out_avals, mesh = _runner()
    args = []
    for n in in_names:
        args.append(per_core[n] if n in _PER_CORE else shared[n])
    zeros = [np.zeros((N_CORES * a.shape[0],) + tuple(a.shape[1:]), a.dtype)
             for a in out_avals]
    return args, zeros


def kernel(**inputs):
    from concourse._compat import axon_active
    if axon_active():
        fn, in_names, out_names, out_avals, mesh = _runner()
        args, zeros = _global_args(inputs)
        outs = fn(*args, *zeros)
        return np.asarray(outs[0])
    # native path (real /dev/neuron*): classic SPMD in_maps
    if "nc" not in _CACHED:
        _CACHED["nc"] = build_nc()
    per_core, shared = _prepare(inputs)
    maps = []
    for c in range(N_CORES):
        m = dict(shared)
        sl = slice(c * B, (c + 1) * B)
        for k, v in per_core.items():
            m[k] = np.ascontiguousarray(v[sl])
        maps.append(m)
    res = bass_utils.run_bass_kernel_spmd(
        _CACHED["nc"], maps, core_ids=list(range(N_CORES)))
    return np.concatenate([r["out"] for r in res.results], axis=0)



# revision 14
# speedup vs baseline: 1.0538x; 1.0538x over previous
"""CLIP vision transformer (prompt tuning variant) on 8 TRN2 NeuronCores.

Data-parallel over batch (64 images -> 8 per core), weights replicated.
Per core the full 12-layer ViT runs on-device:
  - residual stream x: token-major fp32 in SBUF, per-image tiling
    (each image: one 128-row tile + one 77-row tile; token order is
    196 patch tokens, then CLS, then 8 prompt tokens)
  - projections (QKV/O) in float32r (FP22, full PE rate at N>=256)
  - attention QK^T / AV and the whole MLP in bf16 (fp32 psum accum)
  - softmax without max-subtraction (scores are O(1) here); the
    normalizer z comes from an extra ones-column appended per head in V,
    so the AV matmul emits z as psum column 64 -> per-partition
    reciprocal, no cross-partition reduction
  - ln1/ln2 gamma/beta folded into the following projection weights;
    pre/post LN applied via host-prepared broadcast tiles
"""

import numpy as np
import ml_dtypes

import concourse.bass as bass
import concourse.mybir as mybir
import concourse.tile as tile
from concourse import bass_utils
from concourse.vector_clock import ScopedClock

F32 = mybir.dt.float32
F32R = mybir.dt.float32r
BF16 = mybir.dt.bfloat16
AF = mybir.ActivationFunctionType
ALU = mybir.AluOpType
AX = mybir.AxisListType

D, H, DH, L, FF, P, IMG, NPR = 768, 12, 64, 12, 3072, 16, 224, 8
SCALE = 0.125
B_TOT, N_CORES = 64, 8
B = B_TOT // N_CORES          # images per core (8)
S = 205                       # tokens per image
SP = 208                      # padded free-dim stride per image
TF = B * SP                   # feature-major free width (1664)
NPATCH = 196
CLS_ROW = 68                  # cls row within tile 1 (= 196-128)
IMG_TILES = [(0, 128), (1, 77)]
DT = D // 128                 # 6
FT = FF // 128                # 24
VHW = DH + 1                  # v head stride incl. ones column (65)
EPS = 1e-5
TS = bass.ts


# ---------------------------------------------------------------------------
# walrus here accepts at most one semaphore wait per instruction; split
# extras onto same-engine NOPs (and same for the TileContext teardown drain).
# ---------------------------------------------------------------------------
def _split_sync_waits(nc, max_waits=1):
    n = 0
    for f in nc.m.functions:
        for bb in f.blocks:
            out = []
            for inst in bb.instructions:
                si = inst.sync_info
                waits = list(si.on_wait) if si and si.on_wait else []
                if len(waits) > max_waits:
                    for i in range(0, len(waits) - max_waits, max_waits):
                        nop = mybir.InstNoOp(name=f"I-wsplit{n}", ins=[], outs=[])
                        n += 1
                        nop.engine = inst.engine
                        nop.sync_info = mybir.SyncInfo(
                            on_wait=waits[i:i + max_waits], on_update=[])
                        out.append(nop)
                    si.on_wait = waits[len(waits) - max_waits:]
                out.append(inst)
            bb.instructions = out
    return n


def _patched_drain_and_barrier(self, tick_clock, wait_clock):
    nc = self.nc
    drain_inst = nc.sync.drain()
    wait_clock.add_sem_waits(
        drain_inst.ins, ScopedClock({None: tick_clock.global_clock}))
    si = drain_inst.ins.sync_info
    waits = list(si.on_wait or [])
    if len(waits) > 1:
        si.on_wait = waits[:1]
        for i in range(1, len(waits)):
            nop = nc.sync.nop(nofuse=True, hint=f"drain_w{i}")
            nop.ins.sync_info = mybir.SyncInfo(on_wait=[waits[i]], on_update=[])
    nc.all_engine_barrier()
    assert self.sems is not None
    popped = nc._tile_sem_poison_stack.pop()
    assert popped is self._sem_poison
    nc.clear_and_free_semaphores(list(self.sems.allocated().values()))
    nc.all_engine_barrier()


tile.TileContext._drain_and_barrier = _patched_drain_and_barrier


# ---------------------------------------------------------------------------
# host-side input preparation
# ---------------------------------------------------------------------------
def _prepare(inputs):
    f32 = np.float32
    bf = ml_dtypes.bfloat16
    im = np.asarray(inputs["image"], f32)
    gp = np.asarray(inputs["g_prompt"], f32)
    patch_w = np.asarray(inputs["patch_w"], f32)
    cls_emb = np.asarray(inputs["cls_emb"], f32)
    pos_emb = np.asarray(inputs["pos_emb"], f32)

    # im2col -> feature-major patches, padded to SP columns per image
    p = im.reshape(B_TOT, 3, 14, P, 14, P).transpose(0, 2, 4, 1, 3, 5)
    p = p.reshape(B_TOT, NPATCH, 3 * P * P)
    patches_fm = np.zeros((B_TOT, D, SP), bf)
    patches_fm[:, :, :NPATCH] = p.transpose(0, 2, 1).astype(bf)

    patch_wT = patch_w.reshape(D, 3 * P * P).T.astype(bf)      # [768in, 768out]

    extra0 = np.zeros((B_TOT, 1 + NPR, D), f32)
    extra0[:, 0] = cls_emb + pos_emb[0]
    extra0[:, 1:] = gp[:, 0]

    pos_add = np.zeros((2, 128, D), f32)
    pos_add[0] = pos_emb[1:129]
    pos_add[1, :NPATCH - 128] = pos_emb[129:1 + NPATCH]

    pre_gb = np.stack([
        np.broadcast_to(np.asarray(inputs["pre_g"], f32), (128, D)),
        np.broadcast_to(np.asarray(inputs["pre_b"], f32), (128, D)),
    ]).copy()
    post_gb = np.stack([
        np.broadcast_to(np.asarray(inputs["post_g"], f32), (B, D)),
        np.broadcast_to(np.asarray(inputs["post_b"], f32), (B, D)),
    ]).copy()

    qw = np.asarray(inputs["qw"], f32); qb = np.asarray(inputs["qb"], f32)
    kw = np.asarray(inputs["kw"], f32); kb = np.asarray(inputs["kb"], f32)
    vw = np.asarray(inputs["vw"], f32); vb = np.asarray(inputs["vb"], f32)
    ow = np.asarray(inputs["ow"], f32); ob = np.asarray(inputs["ob"], f32)
    g1 = np.asarray(inputs["ln1_g"], f32); b1 = np.asarray(inputs["ln1_b"], f32)
    g2 = np.asarray(inputs["ln2_g"], f32); b2 = np.asarray(inputs["ln2_b"], f32)
    f1w = np.asarray(inputs["fc1_w"], f32); f1b = np.asarray(inputs["fc1_b"], f32)
    f2w = np.asarray(inputs["fc2_w"], f32); f2b = np.asarray(inputs["fc2_b"], f32)

    # fold ln1 into qkv, ln2 into fc1   (w[l,o,d] -> wT[l,d,o])
    qwT = ((qw * g1[:, None, :]).transpose(0, 2, 1) * SCALE).astype(bf)
    kwT = (kw * g1[:, None, :]).transpose(0, 2, 1).astype(bf)
    vwT = (vw * g1[:, None, :]).transpose(0, 2, 1).astype(bf)
    owT = ow.transpose(0, 2, 1).astype(bf)
    f1wT = (f1w * g2[:, None, :]).transpose(0, 2, 1).astype(bf)  # [12,768,3072]
    f2wT = f2w.transpose(0, 2, 1).astype(bf)                     # [12,3072,768]

    qb_eff = (qb + np.einsum("lod,ld->lo", qw, b1)) * SCALE
    kb_eff = kb + np.einsum("lod,ld->lo", kw, b1)
    vb_eff = vb + np.einsum("lod,ld->lo", vw, b1)
    f1b_eff = f1b + np.einsum("lfd,ld->lf", f1w, b2)

    qk_cols = np.concatenate([
        qb_eff.reshape(L, DT, 128).transpose(0, 2, 1),
        kb_eff.reshape(L, DT, 128).transpose(0, 2, 1)], axis=2).copy()
    f1b_cols = f1b_eff.reshape(L, FT, 128).transpose(0, 2, 1).copy()  # [L,128,24]
    bias_rows = np.stack([vb_eff, ob, f2b], axis=1).astype(bf)        # [L, 3, 768]

    per_core = {"patches_fm": patches_fm, "extra0": extra0, "gp": gp}
    shared = {
        "patch_wT": patch_wT, "pos_add": pos_add, "pre_gb": pre_gb,
        "post_gb": post_gb, "qwT": qwT, "kwT": kwT, "vwT": vwT, "owT": owT,
        "f1wT": f1wT, "f2wT": f2wT, "qk_cols": qk_cols,
        "f1b_cols": f1b_cols, "bias_rows": bias_rows,
        "ones_row": np.ones((1, 128), bf), "ident": np.eye(128, dtype=bf),
    }
    return per_core, shared


_DRAM_SPECS = {
    "patches_fm": ((B, D, SP), BF16),
    "extra0": ((B, 1 + NPR, D), F32),
    "gp": ((B, L, NPR, D), F32),
    "patch_wT": ((D, D), BF16),
    "pos_add": ((2, 128, D), F32),
    "pre_gb": ((2, 128, D), F32),
    "post_gb": ((2, B, D), F32),
    "qwT": ((L, D, D), BF16),
    "kwT": ((L, D, D), BF16),
    "vwT": ((L, D, D), BF16),
    "owT": ((L, D, D), BF16),
    "f1wT": ((L, D, FF), BF16),
    "f2wT": ((L, FF, D), BF16),
    "qk_cols": ((L, 128, 2 * DT), F32),
    "f1b_cols": ((L, 128, FT), F32),
    "bias_rows": ((L, 3, D), BF16),
    "ones_row": ((1, 128), BF16),
    "ident": ((128, 128), BF16),
}

N2 = [(0, 512), (512, 256)]    # 768-wide N split


# ---------------------------------------------------------------------------
# device program
# ---------------------------------------------------------------------------
def _ln_stats(nc, sp, x_ap, scratch_ap, nrows, tag):
    """mean/rstd columns ([128,1] tiles, first nrows valid) for rowwise LN."""
    s1 = sp.tile([128, 1], F32, tag=f"{tag}s1", name=f"{tag}s1")
    s2 = sp.tile([128, 1], F32, tag=f"{tag}s2", name=f"{tag}s2")
    mu = sp.tile([128, 1], F32, tag=f"{tag}mu", name=f"{tag}mu")
    var = sp.tile([128, 1], F32, tag=f"{tag}var", name=f"{tag}var")
    rstd = sp.tile([128, 1], F32, tag=f"{tag}rstd", name=f"{tag}rstd")
    r = slice(0, nrows)
    nc.vector.reduce_sum(s1[r, :], x_ap, axis=AX.X)
    nc.scalar.activation(scratch_ap, x_ap, AF.Square, accum_out=s2[r, :])
    nc.vector.tensor_scalar_mul(mu[r, :], s1[r, :], 1.0 / D)
    nc.vector.tensor_mul(var[r, :], mu[r, :], mu[r, :])          # mu^2
    nc.vector.tensor_scalar(s2[r, :], s2[r, :], 1.0 / D, None, op0=ALU.mult)
    nc.vector.tensor_sub(var[r, :], s2[r, :], var[r, :])         # E[x^2]-mu^2
    nc.vector.tensor_scalar(var[r, :], var[r, :], EPS, None, op0=ALU.add)
    nc.scalar.activation(var[r, :], var[r, :], AF.Sqrt)
    nc.vector.reciprocal(rstd[r, :], var[r, :])
    return mu, rstd


def _ln_chunk(nc, lp, sp, psp, x, h_fm, ident, ch, tag, out_dt):
    """LN (stats only) + transpose for one chunk (2 images / 4 tiles).

    Stats for the 4 tiles are batched into [128, 4] column tiles so the
    whole chunk needs one Sqrt + one reciprocal (tames ACT-table swaps
    and tiny-op overhead)."""
    s1 = sp.tile([128, 4], F32, tag=f"{tag}s1", name=f"{tag}s1")
    s2 = sp.tile([128, 4], F32, tag=f"{tag}s2", name=f"{tag}s2")
    mu = sp.tile([128, 4], F32, tag=f"{tag}mu", name=f"{tag}mu")
    var = sp.tile([128, 4], F32, tag=f"{tag}var", name=f"{tag}var")
    rstd = sp.tile([128, 4], F32, tag=f"{tag}rs", name=f"{tag}rs")
    htoks = []
    for t in range(4):
        img, it = ch * 2 + t // 2, t % 2
        i = img * 2 + it
        nrows = IMG_TILES[it][1]
        r = slice(0, nrows)
        htok = lp.tile([128, D], out_dt, tag=f"htok{t}", name=f"htok{t}")
        scr = lp.tile([128, D], F32, tag="hscr", name="hscr")
        nc.vector.reduce_sum(s1[r, t:t + 1], x[i][r, :], axis=AX.X)
        nc.scalar.activation(scr[r, :], x[i][r, :], AF.Square,
                             accum_out=s2[r, t:t + 1])
        htoks.append((htok, i, nrows))
    nc.vector.tensor_scalar_mul(mu[:, :], s1[:, :], 1.0 / D)
    nc.vector.tensor_mul(var[:, :], mu[:, :], mu[:, :])
    nc.vector.tensor_scalar(s2[:, :], s2[:, :], 1.0 / D, None, op0=ALU.mult)
    nc.vector.tensor_sub(var[:, :], s2[:, :], var[:, :])
    nc.vector.tensor_scalar(var[:, :], var[:, :], EPS, None, op0=ALU.add)
    nc.scalar.activation(var[:, :], var[:, :], AF.Sqrt)
    nc.vector.reciprocal(rstd[:, :], var[:, :])
    for t, (htok, i, nrows) in enumerate(htoks):
        r = slice(0, nrows)
        nc.vector.tensor_scalar(htok[r, :], x[i][r, :], mu[r, t:t + 1],
                                rstd[r, t:t + 1], op0=ALU.subtract,
                                op1=ALU.mult)
    for t, (htok, i, nrows) in enumerate(htoks):
        img, it = i // 2, i % 2
        c0 = (img % 2) * SP + it * 128
        for k in range(DT):
            ps = psp.tile([128, 128], out_dt, tag="tp", name="tp")
            nc.tensor.transpose(ps[:, :], htok[:, TS(k, 128)], ident[:, :])
            nc.vector.tensor_copy(h_fm[k][ch][:, c0:c0 + nrows],
                                  ps[:, :nrows])


def _build(nc, tc, dr, out):
    # persistent residual stream: 16 tiles [128, 768] fp32
    xp = tc.alloc_tile_pool(name="x", bufs=1)
    x = [xp.tile([128, D], F32, tag=f"x{i}", name=f"x{i}") for i in range(2 * B)]

    cp = tc.alloc_tile_pool(name="const", bufs=1)
    ident = cp.tile([128, 128], BF16, tag="ident", name="ident")
    ones_r = cp.tile([1, 128], BF16, tag="ones", name="ones")
    nc.sync.dma_start(ident[:, :], dr["ident"][:, :])
    nc.sync.dma_start(ones_r[:, :], dr["ones_row"][:, :])

    # ---------------- embedding ----------------
    with tc.tile_pool(name="emb", bufs=2) as ep, \
         tc.tile_pool(name="embw", bufs=1) as ewp, \
         tc.tile_pool(name="embps", bufs=3, space="PSUM") as psp:
        pw = [ewp.tile([128, D], BF16, tag=f"pw{k}", name=f"pw{k}") for k in range(DT)]
        for k in range(DT):
            nc.sync.dma_start(pw[k][:, :], dr["patch_wT"][TS(k, 128), :])
        pos = [ewp.tile([128, D], F32, tag=f"pos{t}", name=f"pos{t}") for t in range(2)]
        for t in range(2):
            nc.sync.dma_start(pos[t][:, :], dr["pos_add"][t, :, :])
        for img in range(B):
            pat = [ep.tile([128, SP], BF16, tag=f"pat{k}", name=f"pat{k}") for k in range(DT)]
            for k in range(DT):
                nc.sync.dma_start(pat[k][:, :], dr["patches_fm"][img, TS(k, 128), :])
            for it, nrows in IMG_TILES:
                xt = x[img * 2 + it]
                nrp = nrows if it == 0 else NPATCH - 128   # patch rows here
                ps = psp.tile([128, D], F32, tag="embps", name="embps")
                for n0, nw in N2:
                    for k in range(DT):
                        nc.tensor.matmul(
                            ps[:nrp, n0:n0 + nw],
                            pat[k][:, it * 128: it * 128 + nrp],
                            pw[k][:, n0:n0 + nw],
                            start=(k == 0), stop=(k == DT - 1))
                nc.vector.tensor_add(xt[:nrp, :], ps[:nrp, :], pos[it][:nrp, :])
            nc.sync.dma_start(x[img * 2 + 1][CLS_ROW:CLS_ROW + 1 + NPR, :],
                              dr["extra0"][img, :, :])

    # ---------------- pre-layernorm (with real gamma/beta) ----------------
    with tc.tile_pool(name="preln", bufs=3) as pp, \
         tc.tile_pool(name="prest", bufs=1) as sp, \
         tc.tile_pool(name="pregb", bufs=1) as gbp:
        g_bc = gbp.tile([128, D], F32, tag="g_bc", name="g_bc")
        b_bc = gbp.tile([128, D], F32, tag="b_bc", name="b_bc")
        nc.sync.dma_start(g_bc[:, :], dr["pre_gb"][0, :, :])
        nc.sync.dma_start(b_bc[:, :], dr["pre_gb"][1, :, :])
        for i in range(2 * B):
            nrows = IMG_TILES[i % 2][1]
            r = slice(0, nrows)
            scr = pp.tile([128, D], F32, tag="pscr", name="pscr")
            mu, rstd = _ln_stats(nc, sp, x[i][r, :], scr[r, :], nrows, f"p{i}")
            nc.vector.tensor_scalar(x[i][r, :], x[i][r, :], mu[r, :], rstd[r, :],
                                    op0=ALU.subtract, op1=ALU.mult)
            nc.vector.tensor_mul(x[i][r, :], x[i][r, :], g_bc[r, :])
            nc.vector.tensor_add(x[i][r, :], x[i][r, :], b_bc[r, :])

    import os
    nlayers = int(os.environ.get("KBENCH_LAYERS", L))
    for l in range(nlayers):
        _layer(nc, tc, dr, x, ident, ones_r, l)

    # ---------------- final LN on cls rows ----------------
    with tc.tile_pool(name="fin", bufs=1) as fp, \
         tc.tile_pool(name="finst", bufs=1) as fsp:
        cls = fp.tile([B, D], F32, tag="cls", name="cls")
        for img in range(B):
            nc.sync.dma_start(cls[img:img + 1, :],
                              x[img * 2 + 1][CLS_ROW:CLS_ROW + 1, :])
        scr = fp.tile([B, D], F32, tag="fscr", name="fscr")
        mu, rstd = _ln_stats(nc, fsp, cls[:, :], scr[:, :], B, "f")
        nc.vector.tensor_scalar(cls[:, :], cls[:, :], mu[:B, :], rstd[:B, :],
                                op0=ALU.subtract, op1=ALU.mult)
        g_bc = fp.tile([B, D], F32, tag="fg", name="fg")
        b_bc = fp.tile([B, D], F32, tag="fb", name="fb")
        nc.sync.dma_start(g_bc[:, :], dr["post_gb"][0, :, :])
        nc.sync.dma_start(b_bc[:, :], dr["post_gb"][1, :, :])
        nc.vector.tensor_mul(cls[:, :], cls[:, :], g_bc[:, :])
        nc.vector.tensor_add(cls[:, :], cls[:, :], b_bc[:, :])
        nc.sync.dma_start(out[:, :], cls[:, :])
    cp.release()
    xp.release()


def _layer(nc, tc, dr, x, ident, ones_r, l):
    NCH = 4
    CW = TF // NCH             # 416

    if l > 0:
        for img in range(B):
            nc.sync.dma_start(
                x[img * 2 + 1][CLS_ROW + 1:CLS_ROW + 1 + NPR, :],
                dr["gp"][img, l, :, :])

    # layer-long feature-major pool; slots are reused (same tags) for
    # h1_fm -> a_fm -> h2_fm
    fmp = tc.alloc_tile_pool(name=f"fm{l}", bufs=1)
    h_fm = [[fmp.tile([128, CW], BF16, tag=f"hfm{k}_{c}", name=f"hfm{k}_{c}")
             for c in range(NCH)] for k in range(DT)]

    qkvp = tc.alloc_tile_pool(name=f"qkv{l}", bufs=1)
    q_sb = [qkvp.tile([128, TF], BF16, tag=f"q{k}", name=f"q{k}")
            for k in range(DT)]
    k_sb = [qkvp.tile([128, TF], BF16, tag=f"k{k}", name=f"k{k}")
            for k in range(DT)]
    v_sb = [qkvp.tile([128, H * VHW], BF16, tag=f"v{i}", name=f"v{i}")
            for i in range(2 * B)]

    # ---- attention weights: whole layer loaded once ----
    awp = tc.alloc_tile_pool(name=f"aw{l}", bufs=1)
    wq = [awp.tile([128, D], BF16, tag=f"wq{dk}", name=f"wq{dk}")
          for dk in range(DT)]
    wk = [awp.tile([128, D], BF16, tag=f"wk{dk}", name=f"wk{dk}")
          for dk in range(DT)]
    vw = [awp.tile([128, D], BF16, tag=f"vw{dk}", name=f"vw{dk}")
          for dk in range(DT)]
    for dk in range(DT):
        nc.sync.dma_start(wq[dk][:, :], dr["qwT"][l, TS(dk, 128), :])
        nc.sync.dma_start(wk[dk][:, :], dr["kwT"][l, TS(dk, 128), :])
        nc.sync.dma_start(vw[dk][:, :], dr["vwT"][l, TS(dk, 128), :])
    qkb = awp.tile([128, 2 * DT], F32, tag="qkb", name="qkb")
    nc.sync.dma_start(qkb[:, :], dr["qk_cols"][l, :, :])
    vbr = awp.tile([1, D], BF16, tag="vbr", name="vbr")
    nc.sync.dma_start(vbr[:, :], dr["bias_rows"][l, 0:1, :])

    # ======== phase A: LN1 + QKV, chunk-major (PE stays dense) ========
    with tc.tile_pool(name=f"lna{l}", bufs=2) as lp, \
         tc.tile_pool(name=f"lnast{l}", bufs=2) as sp, \
         tc.tile_pool(name=f"tpa{l}", bufs=2, space="PSUM") as tpp, \
         tc.tile_pool(name=f"qkps{l}", bufs=4, space="PSUM") as qkpsp, \
         tc.tile_pool(name=f"vps{l}", bufs=1, space="PSUM") as vpsp:
        for ch in range(NCH):
            _ln_chunk(nc, lp, sp, tpp, x, h_fm, ident, ch, "a", BF16)
            # qk projections for this chunk
            for wsel, wt, bc0 in ((0, wq, 0), (1, wk, DT)):
                sb = q_sb if wsel == 0 else k_sb
                for o in range(DT):
                    ps = qkpsp.tile([128, CW], F32, tag="qkps", name="qkps")
                    for dk in range(DT):
                        nc.tensor.matmul(ps[:, :], wt[dk][:, TS(o, 128)],
                                         h_fm[dk][ch][:, :],
                                         start=(dk == 0), stop=(dk == DT - 1))
                    nc.vector.tensor_scalar(
                        sb[o][:, TS(ch, CW)], ps[:, :],
                        qkb[:, bc0 + o: bc0 + o + 1], None, op0=ALU.add)
            # v projection for the chunk's 4 tiles (token-major out)
            for t in range(4):
                img, it = ch * 2 + t // 2, t % 2
                i = img * 2 + it
                nrows = IMG_TILES[it][1]
                lc = (img % 2) * SP + it * 128
                ps = vpsp.tile([128, D], F32, tag="vps", name="vps")
                for n0, nw in N2:
                    for dk in range(DT):
                        nc.tensor.matmul(ps[:nrows, n0:n0 + nw],
                                         h_fm[dk][ch][:, lc:lc + nrows],
                                         vw[dk][:, n0:n0 + nw],
                                         start=(dk == 0), stop=False)
                    nc.tensor.matmul(ps[:nrows, n0:n0 + nw], ones_r[:, :nrows],
                                     vbr[:, n0:n0 + nw], start=False, stop=True)
                vt = v_sb[i]
                v3 = vt[:nrows, :].rearrange("p (h w) -> p h w", h=H)
                nc.vector.tensor_copy(
                    v3[:, :, :DH],
                    ps[:nrows, :].rearrange("p (h w) -> p h w", h=H))
                nc.vector.memset(v3[:, :, DH:], 1.0)

    # ======== phase B: attention + O + residual, fused per image ========
    # All 24 QK matmuls of an image are emitted back-to-back; head pairs
    # (2hp, 2hp+1) sit in PE row-halves 0:64 / 64:128 so their matmuls run
    # concurrently in the array.  exp/AV/rz trail behind on other engines.
    with tc.tile_pool(name=f"ow{l}", bufs=1) as owp, \
         tc.tile_pool(name=f"att{l}", bufs=2) as apool, \
         tc.tile_pool(name=f"az{l}", bufs=4) as zpool, \
         tc.tile_pool(name=f"asps{l}", bufs=3, space="PSUM") as spsp, \
         tc.tile_pool(name=f"avps{l}", bufs=2, space="PSUM") as apsp, \
         tc.tile_pool(name=f"atps{l}", bufs=1, space="PSUM") as tpsp, \
         tc.tile_pool(name=f"ops{l}", bufs=1, space="PSUM") as opsp:
        a_fm = [[fmp.tile([128, CW], BF16, tag=f"hfm{k}_{c}", name=f"afm{k}_{c}")
                 for c in range(NCH)] for k in range(DT)]
        ow = [owp.tile([128, D], BF16, tag=f"ow{dk}", name=f"ow{dk}")
              for dk in range(DT)]
        for dk in range(DT):
            nc.sync.dma_start(ow[dk][:, :], dr["owT"][l, TS(dk, 128), :])
        obr = owp.tile([1, D], BF16, tag="obr", name="obr")
        nc.sync.dma_start(obr[:, :], dr["bias_rows"][l, 1:2, :])
        ep = tc.alloc_tile_pool(name=f"ae{l}", bufs=1)
        for img in range(B):
            i0 = img * SP
            a_t = [apool.tile([128, D], BF16, tag="a", name="a")
                   for _ in range(2)]
            e_cur = [[None] * 2 for _ in range(H)]
            # -- all QK score matmuls, head pairs adjacent --
            for hp in range(DT):
                for jt, (jti, jw) in enumerate(IMG_TILES):
                    for sub in range(2):
                        h = hp * 2 + sub
                        ko = sub * 64
                        sps = spsp.tile([128, S], F32, tag="sps", name="sps")
                        nc.tensor.matmul(
                            sps[:jw, :],
                            k_sb[hp][ko:ko + DH,
                                     i0 + jti * 128: i0 + jti * 128 + jw],
                            q_sb[hp][ko:ko + DH, i0:i0 + S],
                            start=True, stop=True)
                        e = ep.tile([128, S], BF16, tag=f"e{h}_{jt}",
                                    name=f"e{h}_{jt}")
                        nc.scalar.activation(e[:jw, :], sps[:jw, :], AF.Exp)
                        e_cur[h][jt] = (e, jw)
            # -- all AV matmuls --
            for h in range(H):
                for it, (iti, iw) in enumerate(IMG_TILES):
                    aps = apsp.tile([128, VHW], F32, tag="aps", name="aps")
                    for jt, (e, jw) in enumerate(e_cur[h]):
                        nc.tensor.matmul(
                            aps[:iw, :],
                            e[:jw, iti * 128: iti * 128 + iw],
                            v_sb[img * 2 + jt][:jw, h * VHW: h * VHW + VHW],
                            start=(jt == 0), stop=(jt == 1))
                    rz = zpool.tile([128, 1], F32, tag="rz", name="rz")
                    nc.vector.reciprocal(rz[:iw, :], aps[:iw, DH:DH + 1])
                    nc.vector.tensor_scalar_mul(
                        a_t[it][:iw, TS(h, DH)], aps[:iw, :DH], rz[:iw, :])
            # -- transpose into a_fm, then O projection + residual --
            ch = img // 2
            for it, nrows in IMG_TILES:
                c0 = (img % 2) * SP + it * 128
                for k in range(DT):
                    ps = tpsp.tile([128, 128], BF16, tag="atp", name="atp")
                    nc.tensor.transpose(ps[:, :], a_t[it][:, TS(k, 128)],
                                        ident[:, :])
                    nc.vector.tensor_copy(a_fm[k][ch][:, c0:c0 + nrows],
                                          ps[:, :nrows])
            for it, nrows in IMG_TILES:
                i = img * 2 + it
                lc = (img % 2) * SP + it * 128
                ps = opsp.tile([128, D], F32, tag="ops", name="ops")
                for n0, nw in N2:
                    for dk in range(DT):
                        nc.tensor.matmul(ps[:nrows, n0:n0 + nw],
                                         a_fm[dk][ch][:, lc:lc + nrows],
                                         ow[dk][:, n0:n0 + nw],
                                         start=(dk == 0), stop=False)
                    nc.tensor.matmul(ps[:nrows, n0:n0 + nw], ones_r[:, :nrows],
                                     obr[:, n0:n0 + nw], start=False, stop=True)
                nc.vector.tensor_add(x[i][:nrows, :], x[i][:nrows, :],
                                     ps[:nrows, :])
        ep.release()
    awp.release()
    qkvp.release()

    # ======== phase C: LN2 + MLP, chunk-major ========
    h2_fm = [[fmp.tile([128, CW], BF16, tag=f"hfm{k}_{c}", name=f"h2fm{k}_{c}")
              for c in range(NCH)] for k in range(DT)]
    PCW = SP                   # 208 (one image per hm round)
    with tc.tile_pool(name=f"m1w{l}", bufs=1) as w1p, \
         tc.tile_pool(name=f"m2w{l}", bufs=1) as w2p, \
         tc.tile_pool(name=f"hm{l}", bufs=1) as hmp, \
         tc.tile_pool(name=f"lnc{l}", bufs=2) as lp, \
         tc.tile_pool(name=f"lncst{l}", bufs=2) as sp, \
         tc.tile_pool(name=f"tpc{l}", bufs=1, space="PSUM") as tpp, \
         tc.tile_pool(name=f"m1ps{l}", bufs=3, space="PSUM") as ps1, \
         tc.tile_pool(name=f"m2ps{l}", bufs=2, space="PSUM") as ps2:
        f1b = w1p.tile([128, FT], F32, tag="f1b", name="f1b")
        nc.sync.dma_start(f1b[:, :], dr["f1b_cols"][l, :, :])
        f2br = w2p.tile([1, D], BF16, tag="f2br", name="f2br")
        nc.sync.dma_start(f2br[:, :], dr["bias_rows"][l, 2:3, :])
        f1w = [w1p.tile([128, FF], BF16, tag=f"f1w{dk}", name=f"f1w{dk}")
               for dk in range(DT)]
        for dk in range(DT):
            nc.sync.dma_start(f1w[dk][:, :], dr["f1wT"][l, TS(dk, 128), :])
        f2w = [w2p.tile([128, D], BF16, tag=f"f2w{fk}", name=f"f2w{fk}")
               for fk in range(FT)]
        for fk in range(FT):
            nc.sync.dma_start(f2w[fk][:, :], dr["f2wT"][l, TS(fk, 128), :])
        for ch in range(NCH):
            _ln_chunk(nc, lp, sp, tpp, x, h2_fm, ident, ch, "c", BF16)
            # m1 over the whole chunk (2 images, CW=416 columns per matmul)
            hm = [hmp.tile([128, CW], BF16, tag=f"hm{fk}", name=f"hm{fk}",
                           bufs=2)
                  for fk in range(FT)]
            for fk in range(FT):
                ps = ps1.tile([128, CW], F32, tag="m1ps", name="m1ps")
                for dk in range(DT):
                    nc.tensor.matmul(ps[:, :], f1w[dk][:, TS(fk, 128)],
                                     h2_fm[dk][ch][:, :],
                                     start=(dk == 0), stop=(dk == DT - 1))
                nc.scalar.activation(hm[fk][:, :], ps[:, :],
                                     AF.Gelu_apprx_sigmoid,
                                     bias=f1b[:, fk:fk + 1])
            for t in range(4):
                img, it = ch * 2 + t // 2, t % 2
                nrows = IMG_TILES[it][1]
                loc = (img % 2) * SP + it * 128
                i = img * 2 + it
                ps = ps2.tile([128, D], F32, tag="m2ps", name="m2ps")
                for n0, nw in N2:
                    for fk in range(FT):
                        nc.tensor.matmul(
                            ps[:nrows, n0:n0 + nw],
                            hm[fk][:, loc:loc + nrows],
                            f2w[fk][:, n0:n0 + nw],
                            start=(fk == 0), stop=False)
                    nc.tensor.matmul(ps[:nrows, n0:n0 + nw],
                                     ones_r[:, :nrows],
                                     f2br[:, n0:n0 + nw],
                                     start=False, stop=True)
                nc.vector.tensor_add(x[i][:nrows, :], x[i][:nrows, :],
                                     ps[:nrows, :])
    fmp.release()


def build_nc():
    nc = bass.Bass(trn_type="TRN2", debug=False)
    dr = {name: nc.dram_tensor(name, list(shape), dt, kind="ExternalInput").ap()
          for name, (shape, dt) in _DRAM_SPECS.items()}
    out = nc.dram_tensor("out", [B, D], F32, kind="ExternalOutput").ap()
    with tile.TileContext(nc, pool_alloc_mode="queue") as tc:
        _build(nc, tc, dr, out)
    _split_sync_waits(nc)
    return nc


# ---------------------------------------------------------------------------
# pjrt runner: shared inputs replicated (sent once), per-core inputs sharded
# ---------------------------------------------------------------------------
_PER_CORE = {"patches_fm", "extra0", "gp"}


def _make_runner(nc):
    import jax
    from jax.sharding import Mesh, PartitionSpec
    from jax.experimental.shard_map import shard_map
    from concourse import bass2jax

    bass2jax.install_neuronx_cc_hook()
    partition_name = (nc.partition_id_tensor.name
                      if nc.partition_id_tensor else None)
    in_names, out_names, out_avals = [], [], []
    for alloc in nc.m.functions[0].allocations:
        if not isinstance(alloc, mybir.MemoryLocationSet):
            continue
        name = alloc.memorylocations[0].name
        if alloc.kind == "ExternalInput":
            if name != partition_name:
                in_names.append(name)
        elif alloc.kind == "ExternalOutput":
            out_names.append(name)
            out_avals.append(jax.core.ShapedArray(
                tuple(alloc.tensor_shape), mybir.dt.np(alloc.dtype)))
    n_params = len(in_names)
    n_outs = len(out_names)
    all_names = in_names + out_names
    if partition_name is not None:
        all_names = all_names + [partition_name]

    def _body(*args):
        operands = list(args)
        if partition_name is not None:
            operands.append(bass2jax.partition_id_tensor())
        outs = bass2jax._bass_exec_p.bind(
            *operands, out_avals=tuple(out_avals), in_names=tuple(all_names),
            out_names=tuple(out_names), lowering_input_output_aliases=(),
            sim_require_finite=True, sim_require_nnan=True, nc=nc)
        return tuple(outs)

    devices = jax.devices()[:N_CORES]
    mesh = Mesh(np.asarray(devices), ("core",))
    Pc, Pr = PartitionSpec("core"), PartitionSpec()
    in_specs = tuple(Pc if n in _PER_CORE else Pr for n in in_names) \
        + (Pc,) * n_outs
    out_specs = (Pc,) * n_outs
    fn = jax.jit(
        shard_map(_body, mesh=mesh, in_specs=in_specs, out_specs=out_specs,
                  check_rep=False),
        donate_argnums=tuple(range(n_params, n_params + n_outs)),
        keep_unused=True)
    return fn, in_names, out_names, out_avals, mesh


# ---------------------------------------------------------------------------
# public entry point
# ---------------------------------------------------------------------------
_CACHED = {}


def _runner():
    if "fn" not in _CACHED:
        nc = build_nc()
        _CACHED["fn"] = _make_runner(nc)
    return _CACHED["fn"]


def _global_args(inputs):
    """Argument list for the jitted fn (per-core tensors carry the full
    batch on axis 0; shared tensors passed once)."""
    per_core, shared = _prepare(inputs)
    fn, in_names, out_names, out_avals, mesh = _runner()
    args = []
    for n in in_names:
        args.append(per_core[n] if n in _PER_CORE else shared[n])
    zeros = [np.zeros((N_CORES * a.shape[0],) + tuple(a.shape[1:]), a.dtype)
             for a in out_avals]
    return args, zeros


def kernel(**inputs):
    from concourse._compat import axon_active
    if axon_active():
        fn, in_names, out_names, out_avals, mesh = _runner()
        args, zeros = _global_args(inputs)
        outs = fn(*args, *zeros)
        return np.asarray(outs[0])
    # native path (real /dev/neuron*): classic SPMD in_maps
    if "nc" not in _CACHED:
        _CACHED["nc"] = build_nc()
    per_core, shared = _prepare(inputs)
    maps = []
    for c in range(N_CORES):
        m = dict(shared)
        sl = slice(c * B, (c + 1) * B)
        for k, v in per_core.items():
            m[k] = np.ascontiguousarray(v[sl])
        maps.append(m)
    res = bass_utils.run_bass_kernel_spmd(
        _CACHED["nc"], maps, core_ids=list(range(N_CORES)))
    return np.concatenate([r["out"] for r in res.results], axis=0)



# revision 17
# speedup vs baseline: 1.1562x; 1.0972x over previous
"""CLIP vision transformer (prompt tuning variant) on 8 TRN2 NeuronCores.

Data-parallel over batch (64 images -> 8 per core), weights replicated.
Per core the full 12-layer ViT runs on-device:
  - residual stream x: token-major fp32 in SBUF, per-image tiling
    (each image: one 128-row tile + one 77-row tile; token order is
    196 patch tokens, then CLS, then 8 prompt tokens)
  - projections (QKV/O) in float32r (FP22, full PE rate at N>=256)
  - attention QK^T / AV and the whole MLP in bf16 (fp32 psum accum)
  - softmax without max-subtraction (scores are O(1) here); the
    normalizer z comes from an extra ones-column appended per head in V,
    so the AV matmul emits z as psum column 64 -> per-partition
    reciprocal, no cross-partition reduction
  - ln1/ln2 gamma/beta folded into the following projection weights;
    pre/post LN applied via host-prepared broadcast tiles
"""

import numpy as np
import ml_dtypes

import concourse.bass as bass
import concourse.mybir as mybir
import concourse.tile as tile
from concourse import bass_utils
from concourse.vector_clock import ScopedClock

F32 = mybir.dt.float32
F32R = mybir.dt.float32r
BF16 = mybir.dt.bfloat16
AF = mybir.ActivationFunctionType
ALU = mybir.AluOpType
AX = mybir.AxisListType

D, H, DH, L, FF, P, IMG, NPR = 768, 12, 64, 12, 3072, 16, 224, 8
SCALE = 0.125
B_TOT, N_CORES = 64, 8
B = B_TOT // N_CORES          # images per core (8)
S = 205                       # tokens per image
SP = 208                      # padded free-dim stride per image
TF = B * SP                   # feature-major free width (1664)
NPATCH = 196
CLS_ROW = 68                  # cls row within tile 1 (= 196-128)
IMG_TILES = [(0, 128), (1, 77)]
DT = D // 128                 # 6
FT = FF // 128                # 24
VHW = DH + 1                  # v head stride incl. ones column (65)
EPS = 1e-5
TS = bass.ts


# ---------------------------------------------------------------------------
# walrus here accepts at most one semaphore wait per instruction; split
# extras onto same-engine NOPs (and same for the TileContext teardown drain).
# ---------------------------------------------------------------------------
def _split_sync_waits(nc, max_waits=1):
    n = 0
    for f in nc.m.functions:
        for bb in f.blocks:
            out = []
            for inst in bb.instructions:
                si = inst.sync_info
                waits = list(si.on_wait) if si and si.on_wait else []
                if len(waits) > max_waits:
                    for i in range(0, len(waits) - max_waits, max_waits):
                        nop = mybir.InstNoOp(name=f"I-wsplit{n}", ins=[], outs=[])
                        n += 1
                        nop.engine = inst.engine
                        nop.sync_info = mybir.SyncInfo(
                            on_wait=waits[i:i + max_waits], on_update=[])
                        out.append(nop)
                    si.on_wait = waits[len(waits) - max_waits:]
                out.append(inst)
            bb.instructions = out
    return n


def _patched_drain_and_barrier(self, tick_clock, wait_clock):
    nc = self.nc
    drain_inst = nc.sync.drain()
    wait_clock.add_sem_waits(
        drain_inst.ins, ScopedClock({None: tick_clock.global_clock}))
    si = drain_inst.ins.sync_info
    waits = list(si.on_wait or [])
    if len(waits) > 1:
        si.on_wait = waits[:1]
        for i in range(1, len(waits)):
            nop = nc.sync.nop(nofuse=True, hint=f"drain_w{i}")
            nop.ins.sync_info = mybir.SyncInfo(on_wait=[waits[i]], on_update=[])
    nc.all_engine_barrier()
    assert self.sems is not None
    popped = nc._tile_sem_poison_stack.pop()
    assert popped is self._sem_poison
    nc.clear_and_free_semaphores(list(self.sems.allocated().values()))
    nc.all_engine_barrier()


tile.TileContext._drain_and_barrier = _patched_drain_and_barrier


# ---------------------------------------------------------------------------
# host-side input preparation
# ---------------------------------------------------------------------------
def _prepare(inputs):
    f32 = np.float32
    bf = ml_dtypes.bfloat16
    im = np.asarray(inputs["image"], f32)
    gp = np.asarray(inputs["g_prompt"], f32)
    patch_w = np.asarray(inputs["patch_w"], f32)
    cls_emb = np.asarray(inputs["cls_emb"], f32)
    pos_emb = np.asarray(inputs["pos_emb"], f32)

    # im2col -> feature-major patches, padded to SP columns per image
    p = im.reshape(B_TOT, 3, 14, P, 14, P).transpose(0, 2, 4, 1, 3, 5)
    p = p.reshape(B_TOT, NPATCH, 3 * P * P)
    patches_fm = np.zeros((B_TOT, D, SP), bf)
    patches_fm[:, :, :NPATCH] = p.transpose(0, 2, 1).astype(bf)

    patch_wT = patch_w.reshape(D, 3 * P * P).T.astype(bf)      # [768in, 768out]

    extra0 = np.zeros((B_TOT, 1 + NPR, D), f32)
    extra0[:, 0] = cls_emb + pos_emb[0]
    extra0[:, 1:] = gp[:, 0]

    pos_add = np.zeros((2, 128, D), f32)
    pos_add[0] = pos_emb[1:129]
    pos_add[1, :NPATCH - 128] = pos_emb[129:1 + NPATCH]

    pre_gb = np.stack([
        np.broadcast_to(np.asarray(inputs["pre_g"], f32), (128, D)),
        np.broadcast_to(np.asarray(inputs["pre_b"], f32), (128, D)),
    ]).copy()
    post_gb = np.stack([
        np.broadcast_to(np.asarray(inputs["post_g"], f32), (B, D)),
        np.broadcast_to(np.asarray(inputs["post_b"], f32), (B, D)),
    ]).copy()

    qw = np.asarray(inputs["qw"], f32); qb = np.asarray(inputs["qb"], f32)
    kw = np.asarray(inputs["kw"], f32); kb = np.asarray(inputs["kb"], f32)
    vw = np.asarray(inputs["vw"], f32); vb = np.asarray(inputs["vb"], f32)
    ow = np.asarray(inputs["ow"], f32); ob = np.asarray(inputs["ob"], f32)
    g1 = np.asarray(inputs["ln1_g"], f32); b1 = np.asarray(inputs["ln1_b"], f32)
    g2 = np.asarray(inputs["ln2_g"], f32); b2 = np.asarray(inputs["ln2_b"], f32)
    f1w = np.asarray(inputs["fc1_w"], f32); f1b = np.asarray(inputs["fc1_b"], f32)
    f2w = np.asarray(inputs["fc2_w"], f32); f2b = np.asarray(inputs["fc2_b"], f32)

    # fold ln1 into qkv, ln2 into fc1   (w[l,o,d] -> wT[l,d,o])
    qwT = ((qw * g1[:, None, :]).transpose(0, 2, 1) * SCALE).astype(bf)
    kwT = (kw * g1[:, None, :]).transpose(0, 2, 1).astype(bf)
    vwT = (vw * g1[:, None, :]).transpose(0, 2, 1).astype(bf)
    owT = ow.transpose(0, 2, 1).astype(bf)
    f1wT = (f1w * g2[:, None, :]).transpose(0, 2, 1).astype(bf)  # [12,768,3072]
    f2wT = f2w.transpose(0, 2, 1).astype(bf)                     # [12,3072,768]

    qb_eff = (qb + np.einsum("lod,ld->lo", qw, b1)) * SCALE
    kb_eff = kb + np.einsum("lod,ld->lo", kw, b1)
    vb_eff = vb + np.einsum("lod,ld->lo", vw, b1)
    f1b_eff = f1b + np.einsum("lfd,ld->lf", f1w, b2)

    qk_cols = np.concatenate([
        qb_eff.reshape(L, DT, 128).transpose(0, 2, 1),
        kb_eff.reshape(L, DT, 128).transpose(0, 2, 1)], axis=2).copy()
    f1b_cols = f1b_eff.reshape(L, FT, 128).transpose(0, 2, 1).copy()  # [L,128,24]
    bias_rows = np.stack([vb_eff, ob, f2b], axis=1).astype(bf)        # [L, 3, 768]

    per_core = {"patches_fm": patches_fm, "extra0": extra0, "gp": gp}
    shared = {
        "patch_wT": patch_wT, "pos_add": pos_add, "pre_gb": pre_gb,
        "post_gb": post_gb, "qwT": qwT, "kwT": kwT, "vwT": vwT, "owT": owT,
        "f1wT": f1wT, "f2wT": f2wT, "qk_cols": qk_cols,
        "f1b_cols": f1b_cols, "bias_rows": bias_rows,
        "ones_row": np.ones((1, 128), bf), "ident": np.eye(128, dtype=bf),
    }
    return per_core, shared


_DRAM_SPECS = {
    "patches_fm": ((B, D, SP), BF16),
    "extra0": ((B, 1 + NPR, D), F32),
    "gp": ((B, L, NPR, D), F32),
    "patch_wT": ((D, D), BF16),
    "pos_add": ((2, 128, D), F32),
    "pre_gb": ((2, 128, D), F32),
    "post_gb": ((2, B, D), F32),
    "qwT": ((L, D, D), BF16),
    "kwT": ((L, D, D), BF16),
    "vwT": ((L, D, D), BF16),
    "owT": ((L, D, D), BF16),
    "f1wT": ((L, D, FF), BF16),
    "f2wT": ((L, FF, D), BF16),
    "qk_cols": ((L, 128, 2 * DT), F32),
    "f1b_cols": ((L, 128, FT), F32),
    "bias_rows": ((L, 3, D), BF16),
    "ones_row": ((1, 128), BF16),
    "ident": ((128, 128), BF16),
}

N2 = [(0, 512), (512, 256)]    # 768-wide N split


# ---------------------------------------------------------------------------
# device program
# ---------------------------------------------------------------------------
def _ln_stats(nc, sp, x_ap, scratch_ap, nrows, tag):
    """mean/rstd columns ([128,1] tiles, first nrows valid) for rowwise LN."""
    s1 = sp.tile([128, 1], F32, tag=f"{tag}s1", name=f"{tag}s1")
    s2 = sp.tile([128, 1], F32, tag=f"{tag}s2", name=f"{tag}s2")
    mu = sp.tile([128, 1], F32, tag=f"{tag}mu", name=f"{tag}mu")
    var = sp.tile([128, 1], F32, tag=f"{tag}var", name=f"{tag}var")
    rstd = sp.tile([128, 1], F32, tag=f"{tag}rstd", name=f"{tag}rstd")
    r = slice(0, nrows)
    nc.vector.reduce_sum(s1[r, :], x_ap, axis=AX.X)
    nc.scalar.activation(scratch_ap, x_ap, AF.Square, accum_out=s2[r, :])
    nc.vector.tensor_scalar_mul(mu[r, :], s1[r, :], 1.0 / D)
    nc.vector.tensor_mul(var[r, :], mu[r, :], mu[r, :])          # mu^2
    nc.vector.tensor_scalar(s2[r, :], s2[r, :], 1.0 / D, None, op0=ALU.mult)
    nc.vector.tensor_sub(var[r, :], s2[r, :], var[r, :])         # E[x^2]-mu^2
    nc.vector.tensor_scalar(var[r, :], var[r, :], EPS, None, op0=ALU.add)
    nc.scalar.activation(var[r, :], var[r, :], AF.Sqrt)
    nc.vector.reciprocal(rstd[r, :], var[r, :])
    return mu, rstd


def _ln_chunk(nc, lp, sp, psp, x, h_fm, ident, ch, tag, out_dt):
    """LN (stats only) + transpose for one chunk (2 images / 4 tiles).

    Stats for the 4 tiles are batched into [128, 4] column tiles so the
    whole chunk needs one Sqrt + one reciprocal (tames ACT-table swaps
    and tiny-op overhead)."""
    s1 = sp.tile([128, 4], F32, tag=f"{tag}s1", name=f"{tag}s1")
    s2 = sp.tile([128, 4], F32, tag=f"{tag}s2", name=f"{tag}s2")
    mu = sp.tile([128, 4], F32, tag=f"{tag}mu", name=f"{tag}mu")
    var = sp.tile([128, 4], F32, tag=f"{tag}var", name=f"{tag}var")
    rstd = sp.tile([128, 4], F32, tag=f"{tag}rs", name=f"{tag}rs")
    htoks = []
    for t in range(4):
        img, it = ch * 2 + t // 2, t % 2
        i = img * 2 + it
        nrows = IMG_TILES[it][1]
        r = slice(0, nrows)
        htok = lp.tile([128, D], out_dt, tag=f"htok{t}", name=f"htok{t}")
        scr = lp.tile([128, D], F32, tag="hscr", name="hscr")
        nc.vector.reduce_sum(s1[r, t:t + 1], x[i][r, :], axis=AX.X)
        nc.scalar.activation(scr[r, :], x[i][r, :], AF.Square,
                             accum_out=s2[r, t:t + 1])
        htoks.append((htok, i, nrows))
    nc.vector.tensor_scalar_mul(mu[:, :], s1[:, :], 1.0 / D)
    nc.vector.tensor_mul(var[:, :], mu[:, :], mu[:, :])
    nc.vector.tensor_scalar(s2[:, :], s2[:, :], 1.0 / D, None, op0=ALU.mult)
    nc.vector.tensor_sub(var[:, :], s2[:, :], var[:, :])
    nc.vector.tensor_scalar(var[:, :], var[:, :], EPS, None, op0=ALU.add)
    nc.scalar.activation(var[:, :], var[:, :], AF.Sqrt)
    nc.vector.reciprocal(rstd[:, :], var[:, :])
    for t, (htok, i, nrows) in enumerate(htoks):
        r = slice(0, nrows)
        nc.vector.tensor_scalar(htok[r, :], x[i][r, :], mu[r, t:t + 1],
                                rstd[r, t:t + 1], op0=ALU.subtract,
                                op1=ALU.mult)
    for t, (htok, i, nrows) in enumerate(htoks):
        img, it = i // 2, i % 2
        c0 = (img % 2) * SP + it * 128
        for k in range(DT):
            ps = psp.tile([128, 128], out_dt, tag="tp", name="tp")
            nc.tensor.transpose(ps[:, :], htok[:, TS(k, 128)], ident[:, :])
            nc.vector.tensor_copy(h_fm[k][ch][:, c0:c0 + nrows],
                                  ps[:, :nrows])


def _build(nc, tc, dr, out):
    # persistent residual stream: 16 tiles [128, 768] fp32
    xp = tc.alloc_tile_pool(name="x", bufs=1)
    x = [xp.tile([128, D], F32, tag=f"x{i}", name=f"x{i}") for i in range(2 * B)]

    cp = tc.alloc_tile_pool(name="const", bufs=1)
    ident = cp.tile([128, 128], BF16, tag="ident", name="ident")
    ones_r = cp.tile([1, 128], BF16, tag="ones", name="ones")
    nc.sync.dma_start(ident[:, :], dr["ident"][:, :])
    nc.sync.dma_start(ones_r[:, :], dr["ones_row"][:, :])

    # ---------------- embedding ----------------
    with tc.tile_pool(name="emb", bufs=2) as ep, \
         tc.tile_pool(name="embw", bufs=1) as ewp, \
         tc.tile_pool(name="embps", bufs=3, space="PSUM") as psp:
        pw = [ewp.tile([128, D], BF16, tag=f"pw{k}", name=f"pw{k}") for k in range(DT)]
        for k in range(DT):
            nc.sync.dma_start(pw[k][:, :], dr["patch_wT"][TS(k, 128), :])
        pos = [ewp.tile([128, D], F32, tag=f"pos{t}", name=f"pos{t}") for t in range(2)]
        for t in range(2):
            nc.sync.dma_start(pos[t][:, :], dr["pos_add"][t, :, :])
        for img in range(B):
            pat = [ep.tile([128, SP], BF16, tag=f"pat{k}", name=f"pat{k}") for k in range(DT)]
            for k in range(DT):
                nc.sync.dma_start(pat[k][:, :], dr["patches_fm"][img, TS(k, 128), :])
            for it, nrows in IMG_TILES:
                xt = x[img * 2 + it]
                nrp = nrows if it == 0 else NPATCH - 128   # patch rows here
                ps = psp.tile([128, D], F32, tag="embps", name="embps")
                for n0, nw in N2:
                    for k in range(DT):
                        nc.tensor.matmul(
                            ps[:nrp, n0:n0 + nw],
                            pat[k][:, it * 128: it * 128 + nrp],
                            pw[k][:, n0:n0 + nw],
                            start=(k == 0), stop=(k == DT - 1))
                nc.vector.tensor_add(xt[:nrp, :], ps[:nrp, :], pos[it][:nrp, :])
            nc.sync.dma_start(x[img * 2 + 1][CLS_ROW:CLS_ROW + 1 + NPR, :],
                              dr["extra0"][img, :, :])

    # ---------------- pre-layernorm (with real gamma/beta) ----------------
    with tc.tile_pool(name="preln", bufs=3) as pp, \
         tc.tile_pool(name="prest", bufs=1) as sp, \
         tc.tile_pool(name="pregb", bufs=1) as gbp:
        g_bc = gbp.tile([128, D], F32, tag="g_bc", name="g_bc")
        b_bc = gbp.tile([128, D], F32, tag="b_bc", name="b_bc")
        nc.sync.dma_start(g_bc[:, :], dr["pre_gb"][0, :, :])
        nc.sync.dma_start(b_bc[:, :], dr["pre_gb"][1, :, :])
        for i in range(2 * B):
            nrows = IMG_TILES[i % 2][1]
            r = slice(0, nrows)
            scr = pp.tile([128, D], F32, tag="pscr", name="pscr")
            mu, rstd = _ln_stats(nc, sp, x[i][r, :], scr[r, :], nrows, f"p{i}")
            nc.vector.tensor_scalar(x[i][r, :], x[i][r, :], mu[r, :], rstd[r, :],
                                    op0=ALU.subtract, op1=ALU.mult)
            nc.vector.tensor_mul(x[i][r, :], x[i][r, :], g_bc[r, :])
            nc.vector.tensor_add(x[i][r, :], x[i][r, :], b_bc[r, :])

    import os
    nlayers = int(os.environ.get("KBENCH_LAYERS", L))
    for l in range(nlayers):
        _layer(nc, tc, dr, x, ident, ones_r, l)

    # ---------------- final LN on cls rows ----------------
    with tc.tile_pool(name="fin", bufs=1) as fp, \
         tc.tile_pool(name="finst", bufs=1) as fsp:
        cls = fp.tile([B, D], F32, tag="cls", name="cls")
        for img in range(B):
            nc.sync.dma_start(cls[img:img + 1, :],
                              x[img * 2 + 1][CLS_ROW:CLS_ROW + 1, :])
        scr = fp.tile([B, D], F32, tag="fscr", name="fscr")
        mu, rstd = _ln_stats(nc, fsp, cls[:, :], scr[:, :], B, "f")
        nc.vector.tensor_scalar(cls[:, :], cls[:, :], mu[:B, :], rstd[:B, :],
                                op0=ALU.subtract, op1=ALU.mult)
        g_bc = fp.tile([B, D], F32, tag="fg", name="fg")
        b_bc = fp.tile([B, D], F32, tag="fb", name="fb")
        nc.sync.dma_start(g_bc[:, :], dr["post_gb"][0, :, :])
        nc.sync.dma_start(b_bc[:, :], dr["post_gb"][1, :, :])
        nc.vector.tensor_mul(cls[:, :], cls[:, :], g_bc[:, :])
        nc.vector.tensor_add(cls[:, :], cls[:, :], b_bc[:, :])
        nc.sync.dma_start(out[:, :], cls[:, :])
    cp.release()
    xp.release()


def _layer(nc, tc, dr, x, ident, ones_r, l):
    NCH = 4
    CW = TF // NCH             # 416

    if l > 0:
        for img in range(B):
            nc.sync.dma_start(
                x[img * 2 + 1][CLS_ROW + 1:CLS_ROW + 1 + NPR, :],
                dr["gp"][img, l, :, :])

    # layer-long feature-major pool; slots are reused (same tags) for
    # h1_fm -> a_fm -> h2_fm
    fmp = tc.alloc_tile_pool(name=f"fm{l}", bufs=1)
    h_fm = [[fmp.tile([128, CW], BF16, tag=f"hfm{k}_{c}", name=f"hfm{k}_{c}")
             for c in range(NCH)] for k in range(DT)]

    qkvp = tc.alloc_tile_pool(name=f"qkv{l}", bufs=1)
    q_sb = [qkvp.tile([128, TF], BF16, tag=f"q{k}", name=f"q{k}")
            for k in range(DT)]
    k_sb = [qkvp.tile([128, TF], BF16, tag=f"k{k}", name=f"k{k}")
            for k in range(DT)]
    v_sb = [qkvp.tile([128, H * VHW], BF16, tag=f"v{i}", name=f"v{i}")
            for i in range(2 * B)]

    # ---- attention weights: whole layer loaded once ----
    awp = tc.alloc_tile_pool(name=f"aw{l}", bufs=1)
    wq = [awp.tile([128, D], BF16, tag=f"wq{dk}", name=f"wq{dk}")
          for dk in range(DT)]
    wk = [awp.tile([128, D], BF16, tag=f"wk{dk}", name=f"wk{dk}")
          for dk in range(DT)]
    vw = [awp.tile([128, D], BF16, tag=f"vw{dk}", name=f"vw{dk}")
          for dk in range(DT)]
    for dk in range(DT):
        nc.sync.dma_start(wq[dk][:, :], dr["qwT"][l, TS(dk, 128), :])
        nc.sync.dma_start(wk[dk][:, :], dr["kwT"][l, TS(dk, 128), :])
        nc.sync.dma_start(vw[dk][:, :], dr["vwT"][l, TS(dk, 128), :])
    qkb = awp.tile([128, 2 * DT], F32, tag="qkb", name="qkb")
    nc.sync.dma_start(qkb[:, :], dr["qk_cols"][l, :, :])
    vbr = awp.tile([1, D], BF16, tag="vbr", name="vbr")
    nc.sync.dma_start(vbr[:, :], dr["bias_rows"][l, 0:1, :])

    # ======== phase A: LN1 + QKV, chunk-major (PE stays dense) ========
    with tc.tile_pool(name=f"lna{l}", bufs=2) as lp, \
         tc.tile_pool(name=f"lnast{l}", bufs=2) as sp, \
         tc.tile_pool(name=f"tpa{l}", bufs=2, space="PSUM") as tpp, \
         tc.tile_pool(name=f"qkps{l}", bufs=4, space="PSUM") as qkpsp, \
         tc.tile_pool(name=f"vps{l}", bufs=1, space="PSUM") as vpsp:
        for ch in range(NCH):
            _ln_chunk(nc, lp, sp, tpp, x, h_fm, ident, ch, "a", BF16)
            # qk projections for this chunk
            for wsel, wt, bc0 in ((0, wq, 0), (1, wk, DT)):
                sb = q_sb if wsel == 0 else k_sb
                for o in range(DT):
                    ps = qkpsp.tile([128, CW], F32, tag="qkps", name="qkps")
                    for dk in range(DT):
                        nc.tensor.matmul(ps[:, :], wt[dk][:, TS(o, 128)],
                                         h_fm[dk][ch][:, :],
                                         start=(dk == 0), stop=(dk == DT - 1))
                    nc.vector.tensor_scalar(
                        sb[o][:, TS(ch, CW)], ps[:, :],
                        qkb[:, bc0 + o: bc0 + o + 1], None, op0=ALU.add)
            # v projection for the chunk's 4 tiles (token-major out)
            for t in range(4):
                img, it = ch * 2 + t // 2, t % 2
                i = img * 2 + it
                nrows = IMG_TILES[it][1]
                lc = (img % 2) * SP + it * 128
                ps = vpsp.tile([128, D], F32, tag="vps", name="vps")
                for n0, nw in N2:
                    for dk in range(DT):
                        nc.tensor.matmul(ps[:nrows, n0:n0 + nw],
                                         h_fm[dk][ch][:, lc:lc + nrows],
                                         vw[dk][:, n0:n0 + nw],
                                         start=(dk == 0), stop=False)
                    nc.tensor.matmul(ps[:nrows, n0:n0 + nw], ones_r[:, :nrows],
                                     vbr[:, n0:n0 + nw], start=False, stop=True)
                vt = v_sb[i]
                v3 = vt[:nrows, :].rearrange("p (h w) -> p h w", h=H)
                nc.vector.tensor_copy(
                    v3[:, :, :DH],
                    ps[:nrows, :].rearrange("p (h w) -> p h w", h=H))
                nc.vector.memset(v3[:, :, DH:], 1.0)

    # ======== phase B: attention + O + residual, fused per image ========
    # All 24 QK matmuls of an image are emitted back-to-back; head pairs
    # (2hp, 2hp+1) sit in PE row-halves 0:64 / 64:128 so their matmuls run
    # concurrently in the array.  exp/AV/rz trail behind on other engines.
    with tc.tile_pool(name=f"ow{l}", bufs=1) as owp, \
         tc.tile_pool(name=f"att{l}", bufs=2) as apool, \
         tc.tile_pool(name=f"az{l}", bufs=4) as zpool, \
         tc.tile_pool(name=f"asps{l}", bufs=3, space="PSUM") as spsp, \
         tc.tile_pool(name=f"avps{l}", bufs=2, space="PSUM") as apsp, \
         tc.tile_pool(name=f"atps{l}", bufs=1, space="PSUM") as tpsp, \
         tc.tile_pool(name=f"ops{l}", bufs=1, space="PSUM") as opsp:
        a_fm = [[fmp.tile([128, CW], BF16, tag=f"hfm{k}_{c}", name=f"afm{k}_{c}")
                 for c in range(NCH)] for k in range(DT)]
        ow = [owp.tile([128, D], BF16, tag=f"ow{dk}", name=f"ow{dk}")
              for dk in range(DT)]
        for dk in range(DT):
            nc.sync.dma_start(ow[dk][:, :], dr["owT"][l, TS(dk, 128), :])
        obr = owp.tile([1, D], BF16, tag="obr", name="obr")
        nc.sync.dma_start(obr[:, :], dr["bias_rows"][l, 1:2, :])
        ep = tc.alloc_tile_pool(name=f"ae{l}", bufs=1)
        for img in range(B):
            i0 = img * SP
            a_t = [apool.tile([128, D], BF16, tag="a", name="a")
                   for _ in range(2)]
            e_cur = [[None] * 2 for _ in range(H)]
            # -- all QK score matmuls, head pairs adjacent --
            for hp in range(DT):
                for jt, (jti, jw) in enumerate(IMG_TILES):
                    for sub in range(2):
                        h = hp * 2 + sub
                        ko = sub * 64
                        sps = spsp.tile([128, S], F32, tag="sps", name="sps")
                        nc.tensor.matmul(
                            sps[:jw, :],
                            k_sb[hp][ko:ko + DH,
                                     i0 + jti * 128: i0 + jti * 128 + jw],
                            q_sb[hp][ko:ko + DH, i0:i0 + S],
                            start=True, stop=True)
                        e = ep.tile([128, S], BF16, tag=f"e{h}_{jt}",
                                    name=f"e{h}_{jt}")
                        nc.scalar.activation(e[:jw, :], sps[:jw, :], AF.Exp)
                        e_cur[h][jt] = (e, jw)
            # -- all AV matmuls --
            for h in range(H):
                for it, (iti, iw) in enumerate(IMG_TILES):
                    aps = apsp.tile([128, VHW], F32, tag="aps", name="aps")
                    for jt, (e, jw) in enumerate(e_cur[h]):
                        nc.tensor.matmul(
                            aps[:iw, :],
                            e[:jw, iti * 128: iti * 128 + iw],
                            v_sb[img * 2 + jt][:jw, h * VHW: h * VHW + VHW],
                            start=(jt == 0), stop=(jt == 1))
                    rz = zpool.tile([128, 1], F32, tag="rz", name="rz")
                    nc.vector.reciprocal(rz[:iw, :], aps[:iw, DH:DH + 1])
                    nc.vector.tensor_scalar_mul(
                        a_t[it][:iw, TS(h, DH)], aps[:iw, :DH], rz[:iw, :])
            # -- transpose into a_fm, then O projection + residual --
            ch = img // 2
            for it, nrows in IMG_TILES:
                c0 = (img % 2) * SP + it * 128
                for k in range(DT):
                    ps = tpsp.tile([128, 128], BF16, tag="atp", name="atp")
                    nc.tensor.transpose(ps[:, :], a_t[it][:, TS(k, 128)],
                                        ident[:, :])
                    nc.vector.tensor_copy(a_fm[k][ch][:, c0:c0 + nrows],
                                          ps[:, :nrows])
            for it, nrows in IMG_TILES:
                i = img * 2 + it
                lc = (img % 2) * SP + it * 128
                ps = opsp.tile([128, D], F32, tag="ops", name="ops")
                for n0, nw in N2:
                    for dk in range(DT):
                        nc.tensor.matmul(ps[:nrows, n0:n0 + nw],
                                         a_fm[dk][ch][:, lc:lc + nrows],
                                         ow[dk][:, n0:n0 + nw],
                                         start=(dk == 0), stop=False)
                    nc.tensor.matmul(ps[:nrows, n0:n0 + nw], ones_r[:, :nrows],
                                     obr[:, n0:n0 + nw], start=False, stop=True)
                nc.vector.tensor_add(x[i][:nrows, :], x[i][:nrows, :],
                                     ps[:nrows, :])
        ep.release()
    awp.release()
    qkvp.release()

    # ======== phase C: LN2 + MLP, chunk-major ========
    last = (l == L - 1)
    if not last:
        h2_fm = [[fmp.tile([128, CW], BF16, tag=f"hfm{k}_{c}",
                           name=f"h2fm{k}_{c}")
                  for c in range(NCH)] for k in range(DT)]
    with tc.tile_pool(name=f"m1w{l}", bufs=1) as w1p, \
         tc.tile_pool(name=f"m2w{l}", bufs=1) as w2p, \
         tc.tile_pool(name=f"hm{l}", bufs=1) as hmp, \
         tc.tile_pool(name=f"lnc{l}", bufs=2) as lp, \
         tc.tile_pool(name=f"lncst{l}", bufs=2) as sp, \
         tc.tile_pool(name=f"tpc{l}", bufs=1, space="PSUM") as tpp, \
         tc.tile_pool(name=f"m1ps{l}", bufs=3, space="PSUM") as ps1, \
         tc.tile_pool(name=f"m2ps{l}", bufs=2, space="PSUM") as ps2:
        f1b = w1p.tile([128, FT], F32, tag="f1b", name="f1b")
        nc.sync.dma_start(f1b[:, :], dr["f1b_cols"][l, :, :])
        f2br = w2p.tile([1, D], BF16, tag="f2br", name="f2br")
        nc.sync.dma_start(f2br[:, :], dr["bias_rows"][l, 2:3, :])
        f1w = [w1p.tile([128, FF], BF16, tag=f"f1w{dk}", name=f"f1w{dk}")
               for dk in range(DT)]
        for dk in range(DT):
            nc.sync.dma_start(f1w[dk][:, :], dr["f1wT"][l, TS(dk, 128), :])
        f2w = [w2p.tile([128, D], BF16, tag=f"f2w{fk}", name=f"f2w{fk}")
               for fk in range(FT)]
        for fk in range(FT):
            nc.sync.dma_start(f2w[fk][:, :], dr["f2wT"][l, TS(fk, 128), :])
        if last:
            # Only the CLS rows survive the last layer (prompt rows are
            # dead and patch-token outputs are never read) — run LN2+MLP
            # on the gathered [B, D] CLS batch instead of all 205 tokens.
            xc = lp.tile([B, D], F32, tag="xc", name="xc", bufs=1)
            for img in range(B):
                nc.sync.dma_start(xc[img:img + 1, :],
                                  x[img * 2 + 1][CLS_ROW:CLS_ROW + 1, :])
            scrc = lp.tile([B, D], F32, tag="xcs", name="xcs", bufs=1)
            mu, rstd = _ln_stats(nc, sp, xc[:, :], scrc[:, :], B, "mc")
            h2c = lp.tile([B, D], BF16, tag="h2c", name="h2c", bufs=1)
            nc.vector.tensor_scalar(h2c[:, :], xc[:, :], mu[:B, :],
                                    rstd[:B, :], op0=ALU.subtract,
                                    op1=ALU.mult)
            h2col = [lp.tile([128, B], BF16, tag=f"h2col{k}",
                             name=f"h2col{k}", bufs=1) for k in range(DT)]
            for k in range(DT):
                ps = tpp.tile([128, B], BF16, tag="tpc8", name="tpc8")
                nc.tensor.transpose(ps[:, :], h2c[:, TS(k, 128)],
                                    ident[:B, :B])
                nc.vector.tensor_copy(h2col[k][:, :], ps[:, :])
            hmc = [hmp.tile([128, B], BF16, tag=f"hmc{fk}", name=f"hmc{fk}",
                            bufs=1) for fk in range(FT)]
            for fk in range(FT):
                ps = ps1.tile([128, B], F32, tag="m1ps8", name="m1ps8")
                for dk in range(DT):
                    nc.tensor.matmul(ps[:, :], f1w[dk][:, TS(fk, 128)],
                                     h2col[dk][:, :],
                                     start=(dk == 0), stop=(dk == DT - 1))
                nc.scalar.activation(hmc[fk][:, :], ps[:, :],
                                     AF.Gelu_apprx_sigmoid,
                                     bias=f1b[:, fk:fk + 1])
            ps = ps2.tile([128, D], F32, tag="m2ps", name="m2ps")
            for n0, nw in N2:
                for fk in range(FT):
                    nc.tensor.matmul(ps[:B, n0:n0 + nw], hmc[fk][:, :],
                                     f2w[fk][:, n0:n0 + nw],
                                     start=(fk == 0), stop=False)
                nc.tensor.matmul(ps[:B, n0:n0 + nw], ones_r[:, :B],
                                 f2br[:, n0:n0 + nw], start=False, stop=True)
            nc.vector.tensor_add(xc[:, :], xc[:, :], ps[:B, :])
            for img in range(B):
                nc.sync.dma_start(x[img * 2 + 1][CLS_ROW:CLS_ROW + 1, :],
                                  xc[img:img + 1, :])
        for ch in range(NCH if not last else 0):
            _ln_chunk(nc, lp, sp, tpp, x, h2_fm, ident, ch, "c", BF16)
            # m1 over the whole chunk (2 images, CW=416 columns per matmul)
            hm = [hmp.tile([128, CW], BF16, tag=f"hm{fk}", name=f"hm{fk}",
                           bufs=2)
                  for fk in range(FT)]
            for fk in range(FT):
                ps = ps1.tile([128, CW], F32, tag="m1ps", name="m1ps")
                for dk in range(DT):
                    nc.tensor.matmul(ps[:, :], f1w[dk][:, TS(fk, 128)],
                                     h2_fm[dk][ch][:, :],
                                     start=(dk == 0), stop=(dk == DT - 1))
                nc.scalar.activation(hm[fk][:, :], ps[:, :],
                                     AF.Gelu_apprx_sigmoid,
                                     bias=f1b[:, fk:fk + 1])
            for t in range(4):
                img, it = ch * 2 + t // 2, t % 2
                nrows = IMG_TILES[it][1]
                loc = (img % 2) * SP + it * 128
                i = img * 2 + it
                ps = ps2.tile([128, D], F32, tag="m2ps", name="m2ps")
                for n0, nw in N2:
                    for fk in range(FT):
                        nc.tensor.matmul(
                            ps[:nrows, n0:n0 + nw],
                            hm[fk][:, loc:loc + nrows],
                            f2w[fk][:, n0:n0 + nw],
                            start=(fk == 0), stop=False)
                    nc.tensor.matmul(ps[:nrows, n0:n0 + nw],
                                     ones_r[:, :nrows],
                                     f2br[:, n0:n0 + nw],
                                     start=False, stop=True)
                nc.vector.tensor_add(x[i][:nrows, :], x[i][:nrows, :],
                                     ps[:nrows, :])
    fmp.release()


def build_nc():
    nc = bass.Bass(trn_type="TRN2", debug=False)
    dr = {name: nc.dram_tensor(name, list(shape), dt, kind="ExternalInput").ap()
          for name, (shape, dt) in _DRAM_SPECS.items()}
    out = nc.dram_tensor("out", [B, D], F32, kind="ExternalOutput").ap()
    with tile.TileContext(nc, pool_alloc_mode="queue") as tc:
        _build(nc, tc, dr, out)
    _split_sync_waits(nc)
    return nc


# ---------------------------------------------------------------------------
# pjrt runner: shared inputs replicated (sent once), per-core inputs sharded
# ---------------------------------------------------------------------------
_PER_CORE = {"patches_fm", "extra0", "gp"}


def _make_runner(nc):
    import jax
    from jax.sharding import Mesh, PartitionSpec
    from jax.experimental.shard_map import shard_map
    from concourse import bass2jax

    bass2jax.install_neuronx_cc_hook()
    partition_name = (nc.partition_id_tensor.name
                      if nc.partition_id_tensor else None)
    in_names, out_names, out_avals = [], [], []
    for alloc in nc.m.functions[0].allocations:
        if not isinstance(alloc, mybir.MemoryLocationSet):
            continue
        name = alloc.memorylocations[0].name
        if alloc.kind == "ExternalInput":
            if name != partition_name:
                in_names.append(name)
        elif alloc.kind == "ExternalOutput":
            out_names.append(name)
            out_avals.append(jax.core.ShapedArray(
                tuple(alloc.tensor_shape), mybir.dt.np(alloc.dtype)))
    n_params = len(in_names)
    n_outs = len(out_names)
    all_names = in_names + out_names
    if partition_name is not None:
        all_names = all_names + [partition_name]

    def _body(*args):
        operands = list(args)
        if partition_name is not None:
            operands.append(bass2jax.partition_id_tensor())
        outs = bass2jax._bass_exec_p.bind(
            *operands, out_avals=tuple(out_avals), in_names=tuple(all_names),
            out_names=tuple(out_names), lowering_input_output_aliases=(),
            sim_require_finite=True, sim_require_nnan=True, nc=nc)
        return tuple(outs)

    devices = jax.devices()[:N_CORES]
    mesh = Mesh(np.asarray(devices), ("core",))
    Pc, Pr = PartitionSpec("core"), PartitionSpec()
    in_specs = tuple(Pc if n in _PER_CORE else Pr for n in in_names) \
        + (Pc,) * n_outs
    out_specs = (Pc,) * n_outs
    fn = jax.jit(
        shard_map(_body, mesh=mesh, in_specs=in_specs, out_specs=out_specs,
                  check_rep=False),
        donate_argnums=tuple(range(n_params, n_params + n_outs)),
        keep_unused=True)
    return fn, in_names, out_names, out_avals, mesh


# ---------------------------------------------------------------------------
# public entry point
# ---------------------------------------------------------------------------
_CACHED = {}


def _runner():
    if "fn" not in _CACHED:
        nc = build_nc()
        _CACHED["fn"] = _make_runner(nc)
    return _CACHED["fn"]


def _global_args(inputs):
    """Argument list for the jitted fn (per-core tensors carry the full
    batch on axis 0; shared tensors passed once)."""
    per_core, shared = _prepare(inputs)
    fn, in_names, out_names, out_avals, mesh = _runner()
    args = []
    for n in in_names:
        args.append(per_core[n] if n in _PER_CORE else shared[n])
    zeros = [np.zeros((N_CORES * a.shape[0],) + tuple(a.shape[1:]), a.dtype)
             for a in out_avals]
    return args, zeros


def kernel(**inputs):
    from concourse._compat import axon_active
    if axon_active():
        fn, in_names, out_names, out_avals, mesh = _runner()
        args, zeros = _global_args(inputs)
        outs = fn(*args, *zeros)
        return np.asarray(outs[0])
    # native path (real /dev/neuron*): classic SPMD in_maps
    if "nc" not in _CACHED:
        _CACHED["nc"] = build_nc()
    per_core, shared = _prepare(inputs)
    maps = []
    for c in range(N_CORES):
        m = dict(shared)
        sl = slice(c * B, (c + 1) * B)
        for k, v in per_core.items():
            m[k] = np.ascontiguousarray(v[sl])
        maps.append(m)
    res = bass_utils.run_bass_kernel_spmd(
        _CACHED["nc"], maps, core_ids=list(range(N_CORES)))
    return np.concatenate([r["out"] for r in res.results], axis=0)

